# revision 8
# baseline (speedup 1.0000x reference)
"""Trainium2 Bass kernel for nn_Mixer2dTriUKAN_66417374265858.

Mathematical simplification: in gcn_spatial the adjacency enters only as
s = sum(softmax(P), axis=-1) == 1, so the entire FFT/prob_distance/softmax
branch cancels and gcn_spatial(x, a, w, b) == gelu(x @ (w1+w2+w3).T + b)
where w = [w1|w2|w3] split along the 3T axis.  (Verified: rel err ~9e-7.)

What remains per batch (B=16, C4=128 tokens, T=D=512):
  tm1 = TM(x)   = x + kan64->512(kan512->64(LN(x)))
  y1  = gelu(tm1 @ W1f.T + b1)
  cm  = kan512->512(x)
  tm2 = TM(cm)
  y2  = gelu(tm2 @ W2f.T + b2)
  out = y1 + kan512->512(y2)

kan(x) = silu(x) @ Wb.T + bspl(x) (.) Ws, with the 8 cubic B-spline bases
computed per element as basis_i(x) = (v^3 - 4*relu(v-1)^3)/6 where
v = relu(min(u-i, (i+4)-u)), u = 2.5x + 5.5 -- two fused custom DVE ops
(KAN_VCLAMP 5 ALU stages, KAN_BUMP3 8 stages) per basis.

Sharding: data-parallel over batch, 2 batches per core on 8 cores, weights
replicated.  All activations live in "transposed" layout (feature dim on
partitions, 256 = 2x128 tokens on the free axis); matmuls contract over the
partition axis with bf16 inputs and fp32 PSUM accumulation.
"""
from contextlib import ExitStack

import numpy as np
import ml_dtypes

import concourse.bacc as bacc
import concourse.bass as bass
import concourse.mybir as mybir
import concourse.tile as tile
from concourse.bass import ts
from concourse.bass_utils import run_bass_kernel_spmd
from concourse.masks import make_identity

import concourse.dve_ops as dve_ops
from concourse.dve_ops import DveOp
from concourse.dve_spec import Spec, Src0, Src1, C0, C1, C2, One, relu, sq, minn, lower
from concourse.dve_uop import DveOpSpec

BF = ml_dtypes.bfloat16
F32 = mybir.dt.float32
BF16 = mybir.dt.bfloat16
AF = mybir.ActivationFunctionType

B, C4, T = 16, 128, 512
NCORES = 8
BPC = B // NCORES          # batches per core
NTOK = BPC * C4            # 256 tokens on the free axis
INV_CNT = 1.0 / (C4 * T)   # layernorm element count per batch
EPS = 1e-5
ISQ2 = float(1.0 / np.sqrt(2.0))

_COMPILED = {}             # cache: key -> (nc, input_names)


# --------------------------------------------------------------------------
# custom DVE ops (registered at import, idempotent)
# --------------------------------------------------------------------------
def _mk_op(name, spec, subdim=False):
    shas = {}
    for ver in ("v3", "v4"):
        try:
            s = DveOpSpec(name=name, opcode=0, uops=lower(spec, ver=ver))
            shas[ver] = s.sha(ver)
        except Exception:
            pass
    return DveOp(name, spec, subdim=subdim, uops_sha=shas)


def _register_ops():
    have = {op.name for op in dve_ops.OPS}
    out = {}
    m = Src0 * C0
    _r = relu(Src0 - One)   # shared subexpression: computed once
    specs = {
        # relu(min(x*s0 - s1, imm2 - x*s0))
        "KAN_VCLAMP": Spec(
            body=relu(minn(m - C1, C2 - m)),
            reference=lambda in0, in1, s0, s1, imm2: np.maximum(
                np.minimum(in0 * s0 - s1, imm2 - in0 * s0), 0.0
            ),
        ),
        # v^3 + s0*relu(v-1)^3   (s0=-4)
        "KAN_BUMP3": Spec(
            body=sq(Src0) * Src0 + (sq(_r) * _r) * C0,
            reference=lambda in0, in1, s0, s1, imm2: in0**3
            + s0 * np.maximum(in0 - 1.0, 0.0) ** 3,
        ),
        # (in0*s0) * (in1 + 1)    -- gelu finish: 0.5*h*(1+erf(h/sqrt2))
        "GELU_FIN": Spec(
            body=(Src0 * C0) * (Src1 + One),
            reference=lambda in0, in1, s0, s1, imm2: (in0 * s0) * (in1 + 1.0),
        ),
    }
    for name, spec in specs.items():
        if name in have:
            out[name] = next(op for op in dve_ops.OPS if op.name == name)
            continue
        op = _mk_op(name, spec)
        dve_ops.OPS.append(op)
        dve_ops._SUB_OPCODE_FOR_NAME[name] = (
            dve_ops._CUSTOM_DVE_ROW_BASE + len(dve_ops.OPS) - 1
        )
        dve_ops.CUSTOM_DVE_SPECS[name] = spec
        out[name] = op
    return out


_OPS = _register_ops()
VCLAMP = _OPS["KAN_VCLAMP"]
BUMP3 = _OPS["KAN_BUMP3"]
GELU_FIN = _OPS["GELU_FIN"]


# --------------------------------------------------------------------------
# kernel builder
# --------------------------------------------------------------------------
class _KB:
    """Emission helper holding nc/tc/pools."""

    def __init__(self, nc, tc, ctx):
        self.nc = nc
        self.tc = tc
        p = lambda **kw: ctx.enter_context(tc.tile_pool(**kw))
        self.singles = p(name="singles", bufs=1)
        self.act = p(name="act", bufs=2)        # activation planes (z/cm/y/...)
        self.feat = p(name="feat", bufs=2)      # big bf16 feature buffers
        self.sfeat = p(name="sfeat", bufs=2)    # small (64p) feature buffers
        self.scr = p(name="scr", bufs=2)        # fp32 scratch (v planes, squares)
        self.tiny = p(name="tiny", bufs=8)      # stats vectors
        self.bfa = p(name="bfa", bufs=2)        # bf16 activation planes
        self.psum = p(name="psum", bufs=2, space="PSUM")
        self.psum1 = p(name="psum1", bufs=2, space="PSUM")

        self.ident = self.singles.tile([128, 128], F32)
        make_identity(nc, self.ident[:])
        self.ones = self.singles.tile([128, 128], F32)
        nc.gpsimd.memset(self.ones[:], 1.0)

    # ---- b-spline + silu feature construction --------------------------- #
    def kan_features(self, z, P, W, tag):
        """z: fp32 AP (P, W) flat view.  Returns (feat, sil):
        feat (P, 8, W) bf16 basis planes (x6 scale folded in weights),
        sil  (P, W) bf16 silu(z)."""
        nc = self.nc
        pool = self.feat if P == 128 else self.sfeat
        spool = self.bfa if P == 128 else self.sfeat
        feat = pool.tile([P, 8, W], BF16, tag=f"feat_{128 if P == 128 else 64}")
        sg = self.scr.tile([P, W], F32, tag=f"sg_{128 if P == 128 else 64}")
        nc.scalar.activation(sg[:], z, AF.Sigmoid)
        sil = spool.tile([P, W], BF16, tag=f"sil_{128 if P == 128 else 64}")
        nc.gpsimd.tensor_mul(sil[:], z, sg[:])
        for g in range(8):
            v = self.scr.tile([P, W], F32, tag=f"v_{128 if P == 128 else 64}")
            nc.vector._custom_dve(
                VCLAMP, out=v[:], in0=z, s0=2.5, s1=float(g) - 5.5,
                imm2=float(g) - 1.5,
            )
            nc.vector._custom_dve(BUMP3, out=feat[:, g, :], in0=v[:], s0=-4.0)
        return feat, sil

    # ---- matmul over features ------------------------------------------- #
    def kan_matmul_512(self, feat, sil, w, out_cb):
        """feat (128,8,1024), sil (128,1024), w (128,36,4,128) bf16 lhsT.
        For each m-tile: psum (128,256) after 36 accumulating matmuls ->
        out_cb(m, psum_ap)."""
        nc = self.nc
        for m in range(4):
            pm = self.psum.tile([128, NTOK], F32, tag="pmm")
            n = 0
            for g in range(9):
                for k in range(4):
                    rhs = sil[:, ts(k, NTOK)] if g == 8 else feat[:, g, ts(k, NTOK)]
                    nc.tensor.matmul(
                        pm[:], w[:, g * 4 + k, m, :], rhs,
                        start=(n == 0), stop=(n == 35),
                    )
                    n += 1
            out_cb(m, pm)

    def kan_matmul_512_to_64(self, feat, sil, w):
        """-> psum (64, 256) after 36 matmuls. w (128, 36, 64)."""
        nc = self.nc
        pm = self.psum1.tile([64, NTOK], F32, tag="pk64")
        n = 0
        for g in range(9):
            for k in range(4):
                rhs = sil[:, ts(k, NTOK)] if g == 8 else feat[:, g, ts(k, NTOK)]
                nc.tensor.matmul(
                    pm[:], w[:, g * 4 + k, :], rhs, start=(n == 0), stop=(n == 35)
                )
                n += 1
        return pm

    def kan_matmul_64_to_512(self, feat, sil, w, out_cb):
        """feat (64,8,256), sil (64,256), w (64,9,4,128)."""
        nc = self.nc
        for m in range(4):
            pm = self.psum.tile([128, NTOK], F32, tag="pmm")
            for g in range(9):
                rhs = sil[:] if g == 8 else feat[:, g, :]
                nc.tensor.matmul(
                    pm[:], w[:, g, m, :], rhs, start=(g == 0), stop=(g == 8)
                )
            out_cb(m, pm)

    # ---- layernorm ------------------------------------------------------ #
    def layernorm(self, xT, zname, lnw=None, lnb=None):
        """xT (128, 4, NTOK) fp32 -> z (128, 4, NTOK) fp32 normalized
        per batch over all (T x C4) elements."""
        nc = self.nc
        stats = self.tiny.tile([128, 2 * BPC], F32)  # [s_b, ss_b]*BPC
        for b in range(BPC):
            sl = xT[:, :, ts(b, C4)]
            nc.vector.tensor_reduce(
                out=stats[:, 2 * b : 2 * b + 1], in_=sl,
                op=mybir.AluOpType.add, axis=mybir.AxisListType.XY,
            )
            sqr = self.scr.tile([128, 4, C4], F32, tag="sqscr")
            nc.scalar.activation(
                sqr[:], sl, AF.Square, accum_out=stats[:, 2 * b + 1 : 2 * b + 2]
            )
        pstat = self.psum1.tile([128, 2 * BPC], F32, tag="pstat")
        nc.tensor.matmul(pstat[:], self.ones[:], stats[:], start=True, stop=True)
        statsF = self.tiny.tile([128, 2 * BPC], F32)
        nc.vector.tensor_scalar(
            out=statsF[:], in0=pstat[:], scalar1=INV_CNT, scalar2=None,
            op0=mybir.AluOpType.mult,
        )
        mu = statsF[:, 0 : 2 * BPC : 2]
        e2 = statsF[:, 1 : 2 * BPC : 2]
        var = self.tiny.tile([128, BPC], F32)
        nc.vector.tensor_mul(var[:], mu, mu)
        nc.vector.tensor_sub(var[:], e2, var[:])
        a = self.tiny.tile([128, BPC], F32)
        nc.vector.tensor_scalar_add(a[:], var[:], EPS)
        # y = rsqrt(a) by Newton from y0 = min(1/a, 1) (monotone convergence)
        y = self.tiny.tile([128, BPC], F32)
        nc.vector.reciprocal(y[:], a[:])
        nc.vector.tensor_scalar_min(y[:], y[:], 1.0)
        t = self.tiny.tile([128, BPC], F32)
        for _ in range(10):
            nc.vector.tensor_mul(t[:], y[:], y[:])
            nc.vector.tensor_mul(t[:], t[:], a[:])
            nc.vector.tensor_scalar(
                out=t[:], in0=t[:], scalar1=-0.5, scalar2=1.5,
                op0=mybir.AluOpType.mult, op1=mybir.AluOpType.add,
            )
            nc.vector.tensor_mul(y[:], y[:], t[:])
        musc = self.tiny.tile([128, BPC], F32)
        nc.vector.tensor_mul(musc[:], mu, y[:])
        z = self.act.tile([128, 4, NTOK], F32, tag=zname)
        for b in range(BPC):
            nc.vector.tensor_scalar(
                out=z[:, :, ts(b, C4)], in0=xT[:, :, ts(b, C4)],
                scalar1=y[:, b : b + 1], scalar2=musc[:, b : b + 1],
                op0=mybir.AluOpType.mult, op1=mybir.AluOpType.subtract,
            )
        if lnw is not None:
            nc.vector.tensor_mul(z[:], z[:], lnw[:])
        if lnb is not None:
            nc.vector.tensor_add(z[:], z[:], lnb[:])
        return z

    # ---- gcn (folded) ---------------------------------------------------- #
    def gcn(self, tm_bf, wg, bias, bias_sc, yname):
        """tm_bf (128,4,NTOK) bf16; wg (128,4,4,128) bf16; bias (128,4) f32.
        Returns y (128,4,NTOK) f32 = gelu(tm @ Wg + b)."""
        nc = self.nc
        y = self.act.tile([128, 4, NTOK], F32, tag=yname)
        for m in range(4):
            pm = self.psum.tile([128, NTOK], F32, tag="pmm")
            for k in range(4):
                nc.tensor.matmul(
                    pm[:], wg[:, k, m, :], tm_bf[:, k, :],
                    start=(k == 0), stop=(k == 3),
                )
            hb = self.scr.tile([128, NTOK], F32, tag="hb")
            nc.scalar.activation(hb[:], pm[:], AF.Identity, bias=bias[:, m : m + 1])
            e = self.scr.tile([128, NTOK], F32, tag="erf")
            nc.scalar.activation(
                e[:], pm[:], AF.Erf, bias=bias_sc[:, m : m + 1], scale=ISQ2
            )
            nc.vector._custom_dve(
                GELU_FIN, out=y[:, m, :], in0=hb[:], in1=e[:], s0=0.5
            )
        return y


def _emit(nc, ln_flags):
    """Emit the full per-core kernel.  ln_flags = (use_lnw1, use_lnb1,
    use_lnw2, use_lnb2) -- whether the TM layernorm affine params are
    non-trivial and must be applied."""
    use_lnw1, use_lnb1, use_lnw2, use_lnb2 = ln_flags
    dram = {}

    def din(name, shape, dt=BF16):
        dram[name] = nc.dram_tensor(name, shape, dt, kind="ExternalInput").ap()
        return dram[name]

    x_d = din("x_sh", (BPC, C4, T), F32)
    w_tm1k1 = din("w_tm1k1", (128, 36, 64))
    w_tm1k2 = din("w_tm1k2", (64, 9, 4, 128))
    w_k1 = din("w_k1", (128, 36, 4, 128))
    w_g1 = din("w_g1", (128, 4, 4, 128))
    b_g1 = din("b_g1", (128, 4, 2), F32)        # [:, :, 0]=b, [:, :, 1]=b/sqrt2
    w_tm2k1 = din("w_tm2k1", (128, 36, 64))
    w_tm2k2 = din("w_tm2k2", (64, 9, 4, 128))
    w_g2 = din("w_g2", (128, 4, 4, 128))
    b_g2 = din("b_g2", (128, 4, 2), F32)
    w_k2 = din("w_k2", (128, 36, 4, 128))
    ln1w_d = din("ln1w", (128, 4, NTOK), F32) if use_lnw1 else None
    ln1b_d = din("ln1b", (128, 4, NTOK), F32) if use_lnb1 else None
    ln2w_d = din("ln2w", (128, 4, NTOK), F32) if use_lnw2 else None
    ln2b_d = din("ln2b", (128, 4, NTOK), F32) if use_lnb2 else None
    out_d = nc.dram_tensor("out_sh", (BPC, C4, T), F32, kind="ExternalOutput").ap()

    with tile.TileContext(nc) as tc, ExitStack() as ctx:
        kb = _KB(nc, tc, ctx)
        wpool = ctx.enter_context(tc.tile_pool(name="weights", bufs=1))
        sync = nc.sync

        # ---- weight/input DMA (issue in consumption order) ----
        def wload(ap, shape, tag, dt=BF16):
            t = wpool.tile(list(shape), dt, tag=tag)
            sync.dma_start(t[:], ap)
            return t

        xN = kb.act.tile([C4, BPC, T], F32, tag="nat")
        sync.dma_start(xN[:], x_d.rearrange("b p t -> p b t"))
        W_k1 = wload(w_k1, (128, 36, 4, 128), "wk_big")
        W_tm1k1 = wload(w_tm1k1, (128, 36, 64), "wk_top")
        W_tm1k2 = wload(w_tm1k2, (64, 9, 4, 128), "wk_bot")
        W_g1 = wload(w_g1, (128, 4, 4, 128), "wg")
        B_g1 = wload(b_g1, (128, 4, 2), "bg1", F32)
        W_tm2k1 = wload(w_tm2k1, (128, 36, 64), "wk_top")
        W_tm2k2 = wload(w_tm2k2, (64, 9, 4, 128), "wk_bot")
        W_g2 = wload(w_g2, (128, 4, 4, 128), "wg")
        B_g2 = wload(b_g2, (128, 4, 2), "bg2", F32)
        W_k2 = wload(w_k2, (128, 36, 4, 128), "wk_big")
        LN1W = wload(ln1w_d, (128, 4, NTOK), "ln1w", F32) if use_lnw1 else None
        LN1B = wload(ln1b_d, (128, 4, NTOK), "ln1b", F32) if use_lnb1 else None
        LN2W = wload(ln2w_d, (128, 4, NTOK), "ln2w", F32) if use_lnw2 else None
        LN2B = wload(ln2b_d, (128, 4, NTOK), "ln2b", F32) if use_lnb2 else None

        # ---- transpose x into T-layout ----
        xT = kb.act.tile([128, 4, NTOK], F32, tag="xT")
        for k in range(4):
            for b in range(BPC):
                pt = kb.psum.tile([128, 128], F32, tag="ptr")
                nc.tensor.transpose(pt[:], xN[:, b, ts(k, 128)], kb.ident[:])
                nc.scalar.copy(xT[:, k, ts(b, 128)], pt[:])

        # ---- branch A head: k1 features (longest dependency chain) ----
        featX, silX = kb.kan_features(xT[:, :, :], 128, 4 * NTOK, "x")

        # ---- TM1: layernorm + kan1 features ----
        z1 = kb.layernorm(xT, "z", LN1W, LN1B)
        featA, silA = kb.kan_features(z1[:, :, :], 128, 4 * NTOK, "a")

        # ---- k1 matmuls -> cm ----
        cm = kb.act.tile([128, 4, NTOK], F32, tag="cm")
        kb.kan_matmul_512(
            featX, silX, W_k1,
            lambda m, pm: nc.scalar.copy(cm[:, m, :], pm[:]),
        )

        # ---- TM1 kan1 -> z2, kan2 -> tm1 ----
        p1 = kb.kan_matmul_512_to_64(featA, silA, W_tm1k1)
        z2 = kb.act.tile([64, NTOK], F32, tag="z64")
        nc.scalar.copy(z2[:], p1[:])
        featB, silB = kb.kan_features(z2[:, :], 64, NTOK, "b")
        tm1 = kb.bfa.tile([128, 4, NTOK], BF16, tag="tm")
        kb.kan_matmul_64_to_512(
            featB, silB, W_tm1k2,
            lambda m, pm: nc.vector.tensor_add(tm1[:, m, :], xT[:, m, :], pm[:]),
        )

        # ---- TM2 on cm ----
        z3 = kb.layernorm(cm, "z", LN2W, LN2B)
        featC, silC = kb.kan_features(z3[:, :, :], 128, 4 * NTOK, "c")
        p2 = kb.kan_matmul_512_to_64(featC, silC, W_tm2k1)
        z4 = kb.act.tile([64, NTOK], F32, tag="z64")
        nc.scalar.copy(z4[:], p2[:])
        featD, silD = kb.kan_features(z4[:, :], 64, NTOK, "d")
        tm2 = kb.bfa.tile([128, 4, NTOK], BF16, tag="tm")
        kb.kan_matmul_64_to_512(
            featD, silD, W_tm2k2,
            lambda m, pm: nc.vector.tensor_add(tm2[:, m, :], cm[:, m, :], pm[:]),
        )

        # ---- gcn1 (y1) and gcn2 (y2) ----
        y1 = kb.gcn(tm1, W_g1, B_g1[:, :, 0], B_g1[:, :, 1], "y")
        y2 = kb.gcn(tm2, W_g2, B_g2[:, :, 0], B_g2[:, :, 1], "y")

        # ---- k2 on y2, final add, transpose out ----
        featY, silY = kb.kan_features(y2[:, :, :], 128, 4 * NTOK, "y")
        outT = kb.act.tile([128, 4, NTOK], F32, tag="outT")
        kb.kan_matmul_512(
            featY, silY, W_k2,
            lambda m, pm: nc.vector.tensor_add(outT[:, m, :], y1[:, m, :], pm[:]),
        )

        outN = kb.act.tile([C4, BPC, T], F32, tag="nat")
        for m in range(4):
            for b in range(BPC):
                pt = kb.psum.tile([128, 128], F32, tag="ptr")
                nc.tensor.transpose(pt[:], outT[:, m, ts(b, 128)], kb.ident[:])
                nc.scalar.copy(outN[:, b, ts(m, 128)], pt[:])
        sync.dma_start(out_d.rearrange("b p t -> p b t"), outN[:])

    return dram


def _build(ln_flags):
    key = ln_flags
    if key in _COMPILED:
        return _COMPILED[key]
    nc = bacc.Bacc("TRN2", target_bir_lowering=False, debug=False)
    _emit(nc, ln_flags)
    nc.compile()
    _COMPILED[key] = nc
    return nc


# --------------------------------------------------------------------------
# host-side weight preparation
# --------------------------------------------------------------------------
def _prep_kan_512(base_w, spline_w):
    """base_w (O,512), spline_w (O,512,8) -> (128, 36, O) or (128,36,4,128)."""
    O = base_w.shape[0]
    w = np.empty((128, 36, O), np.float32)
    for g in range(8):
        for k in range(4):
            # rows p -> channel 128k+p, feature basis g (x 1/6)
            w[:, g * 4 + k, :] = spline_w[:, k * 128 : (k + 1) * 128, g].T / 6.0
    for k in range(4):
        w[:, 32 + k, :] = base_w[:, k * 128 : (k + 1) * 128].T
    w = w.astype(BF)
    if O == 512:
        return np.ascontiguousarray(w.reshape(128, 36, 4, 128))
    return np.ascontiguousarray(w)


def _prep_kan_64(base_w, spline_w):
    """base_w (512,64), spline_w (512,64,8) -> (64, 9, 4, 128)."""
    w = np.empty((64, 9, 4, 128), np.float32)
    for g in range(8):
        for m in range(4):
            w[:, g, m, :] = spline_w[m * 128 : (m + 1) * 128, :, g].T / 6.0
    for m in range(4):
        w[:, 8, m, :] = base_w[m * 128 : (m + 1) * 128, :].T
    return np.ascontiguousarray(w.astype(BF))


def _prep_gcn(gw, gb):
    """gw (512, 1536) -> folded (128,4,4,128) bf16 lhsT; gb -> (128,4,2) f32."""
    Wf = gw[:, :512] + gw[:, 512:1024] + gw[:, 1024:]
    w = np.empty((128, 4, 4, 128), np.float32)
    for k in range(4):
        for m in range(4):
            w[:, k, m, :] = Wf[m * 128 : (m + 1) * 128, k * 128 : (k + 1) * 128].T
    b = np.empty((128, 4, 2), np.float32)
    b[:, :, 0] = gb.reshape(4, 128).T
    b[:, :, 1] = b[:, :, 0] * ISQ2
    return np.ascontiguousarray(w.astype(BF)), np.ascontiguousarray(b)


def _ln_plane(a):
    """ln param (512, 128) -> (128, 4, NTOK) f32 duplicated over batches."""
    p = np.empty((128, 4, NTOK), np.float32)
    for k in range(4):
        for b in range(BPC):
            p[:, k, b * C4 : (b + 1) * C4] = a[k * 128 : (k + 1) * 128, :]
    return np.ascontiguousarray(p)


def kernel(**inputs):
    i = {k: np.asarray(v) for k, v in inputs.items()}
    use_lnw1 = not np.all(i["tm1_ln_w"] == 1.0)
    use_lnb1 = not np.all(i["tm1_ln_b"] == 0.0)
    use_lnw2 = not np.all(i["tm_ln_w"] == 1.0)
    use_lnb2 = not np.all(i["tm_ln_b"] == 0.0)
    ln_flags = (use_lnw1, use_lnb1, use_lnw2, use_lnb2)
    nc = _build(ln_flags)

    w_tm1k1 = _prep_kan_512(i["tm1_k1_base"], i["tm1_k1_spline"])
    w_tm1k2 = _prep_kan_64(i["tm1_k2_base"], i["tm1_k2_spline"])
    w_k1 = _prep_kan_512(i["k1_base"], i["k1_spline"])
    w_g1, b_g1 = _prep_gcn(i["g1_w"], i["g1_b"])
    w_tm2k1 = _prep_kan_512(i["tm_k1_base"], i["tm_k1_spline"])
    w_tm2k2 = _prep_kan_64(i["tm_k2_base"], i["tm_k2_spline"])
    w_g2, b_g2 = _prep_gcn(i["g2_w"], i["g2_b"])
    w_k2 = _prep_kan_512(i["k2_base"], i["k2_spline"])

    shared = dict(
        w_tm1k1=w_tm1k1, w_tm1k2=w_tm1k2, w_k1=w_k1, w_g1=w_g1, b_g1=b_g1,
        w_tm2k1=w_tm2k1, w_tm2k2=w_tm2k2, w_g2=w_g2, b_g2=b_g2, w_k2=w_k2,
    )
    if use_lnw1:
        shared["ln1w"] = _ln_plane(i["tm1_ln_w"])
    if use_lnb1:
        shared["ln1b"] = _ln_plane(i["tm1_ln_b"])
    if use_lnw2:
        shared["ln2w"] = _ln_plane(i["tm_ln_w"])
    if use_lnb2:
        shared["ln2b"] = _ln_plane(i["tm_ln_b"])
    x = np.ascontiguousarray(i["x"], np.float32)
    in_maps = [
        {"x_sh": x[c * BPC : (c + 1) * BPC], **shared} for c in range(NCORES)
    ]
    res = run_bass_kernel_spmd(nc, in_maps, core_ids=list(range(NCORES)))
    out = np.empty((B, C4, T), np.float32)
    for c in range(NCORES):
        out[c * BPC : (c + 1) * BPC] = res.results[c]["out_sh"]
    return out


# revision 24
# speedup vs baseline: 1.1818x; 1.1818x over previous
"""Trainium2 Bass kernel for nn_Mixer2dTriUKAN_66417374265858.

Mathematical simplification: in gcn_spatial the adjacency enters only as
s = sum(softmax(P), axis=-1) == 1, so the entire FFT/prob_distance/softmax
branch cancels and gcn_spatial(x, a, w, b) == gelu(x @ (w1+w2+w3).T + b)
where w = [w1|w2|w3] split along the 3T axis.  (Verified: rel err ~9e-7.)

What remains per batch (B=16, C4=128 tokens, T=D=512):
  tm1 = TM(x)   = x + kan64->512(kan512->64(LN(x)))
  y1  = gelu(tm1 @ W1f.T + b1)
  cm  = kan512->512(x)
  tm2 = TM(cm)
  y2  = gelu(tm2 @ W2f.T + b2)
  out = y1 + kan512->512(y2)

kan(x) = silu(x) @ Wb.T + bspl(x) (.) Ws, with the 8 cubic B-spline bases
computed per element as basis_i(x) = (v^3 - 4*relu(v-1)^3)/6 where
v = relu(min(u-i, (i+4)-u)), u = 2.5x + 5.5 -- two fused custom DVE ops
(KAN_VCLAMP 5 ALU stages, KAN_BUMP3 8 stages) per basis.

Sharding: data-parallel over batch, 2 batches per core on 8 cores, weights
replicated.  All activations live in "transposed" layout (feature dim on
partitions, 256 = 2x128 tokens on the free axis); matmuls contract over the
partition axis with bf16 inputs and fp32 PSUM accumulation.
"""
from contextlib import ExitStack

import numpy as np
import ml_dtypes

import concourse.bacc as bacc
import concourse.bass as bass
import concourse.mybir as mybir
import concourse.tile as tile
from concourse.bass import ts
from concourse.bass_utils import run_bass_kernel_spmd
from concourse.masks import make_identity

import concourse.dve_ops as dve_ops
from concourse.dve_ops import DveOp
from concourse.dve_spec import Spec, Src0, Src1, C0, C1, C2, One, relu, sq, minn, lower
from concourse.dve_uop import DveOpSpec

BF = ml_dtypes.bfloat16
F32 = mybir.dt.float32
BF16 = mybir.dt.bfloat16
AF = mybir.ActivationFunctionType

B, C4, T = 16, 128, 512
NCORES = 8
BPC = B // NCORES          # batches per core
NTOK = BPC * C4            # 256 tokens on the free axis
INV_CNT = 1.0 / (C4 * T)   # layernorm element count per batch
EPS = 1e-5
ISQ2 = float(1.0 / np.sqrt(2.0))

_COMPILED = {}             # cache: key -> (nc, input_names)


# --------------------------------------------------------------------------
# custom DVE ops (registered at import, idempotent)
# --------------------------------------------------------------------------
def _mk_op(name, spec, subdim=False):
    shas = {}
    for ver in ("v3", "v4"):
        try:
            s = DveOpSpec(name=name, opcode=0, uops=lower(spec, ver=ver))
            shas[ver] = s.sha(ver)
        except Exception:
            pass
    return DveOp(name, spec, subdim=subdim, uops_sha=shas)


def _register_ops():
    have = {op.name for op in dve_ops.OPS}
    out = {}
    m = Src0 * C0
    _r = relu(Src0 - One)   # shared subexpression: computed once
    specs = {
        # relu(min(x*s0 - s1, imm2 - x*s0))
        "KAN_VCLAMP": Spec(
            body=relu(minn(m - C1, C2 - m)),
            reference=lambda in0, in1, s0, s1, imm2: np.maximum(
                np.minimum(in0 * s0 - s1, imm2 - in0 * s0), 0.0
            ),
        ),
        # v^3 + s0*relu(v-1)^3   (s0=-4)
        "KAN_BUMP3": Spec(
            body=sq(Src0) * Src0 + (sq(_r) * _r) * C0,
            reference=lambda in0, in1, s0, s1, imm2: in0**3
            + s0 * np.maximum(in0 - 1.0, 0.0) ** 3,
        ),
        # (in0*s0) * (in1 + 1)    -- gelu finish: 0.5*h*(1+erf(h/sqrt2))
        "GELU_FIN": Spec(
            body=(Src0 * C0) * (Src1 + One),
            reference=lambda in0, in1, s0, s1, imm2: (in0 * s0) * (in1 + 1.0),
        ),
    }
    for name, spec in specs.items():
        if name in have:
            out[name] = next(op for op in dve_ops.OPS if op.name == name)
            continue
        op = _mk_op(name, spec)
        dve_ops.OPS.append(op)
        dve_ops._SUB_OPCODE_FOR_NAME[name] = (
            dve_ops._CUSTOM_DVE_ROW_BASE + len(dve_ops.OPS) - 1
        )
        dve_ops.CUSTOM_DVE_SPECS[name] = spec
        out[name] = op
    return out


_OPS = _register_ops()
VCLAMP = _OPS["KAN_VCLAMP"]
BUMP3 = _OPS["KAN_BUMP3"]
GELU_FIN = _OPS["GELU_FIN"]


# --------------------------------------------------------------------------
# kernel builder
# --------------------------------------------------------------------------
class _KB:
    """Emission helper holding nc/tc/pools."""

    def __init__(self, nc, tc, ctx):
        self.nc = nc
        self.tc = tc
        p = lambda **kw: ctx.enter_context(tc.tile_pool(**kw))
        self.singles = p(name="singles", bufs=1)
        self.act = p(name="act", bufs=2)        # activation planes (z/cm/y/...)
        self.feat = p(name="feat", bufs=2)      # big bf16 feature buffers
        self.sfeat = p(name="sfeat", bufs=2)    # small (64p) feature buffers
        self.scr = p(name="scr", bufs=2)        # fp32 scratch (v planes, squares)
        self.tiny = p(name="tiny", bufs=8)      # stats vectors
        self.bfa = p(name="bfa", bufs=2)        # bf16 activation planes
        self.psum4 = p(name="psum4", bufs=5, space="PSUM")
        self.psum = p(name="psum", bufs=2, space="PSUM")
        self.psum1 = p(name="psum1", bufs=1, space="PSUM")

        self.ident = self.singles.tile([128, 128], F32)
        make_identity(nc, self.ident[:])
        self.ones = self.singles.tile([128, 128], F32)
        nc.gpsimd.memset(self.ones[:], 1.0)

    # ---- b-spline + silu feature construction --------------------------- #
    def kan_features(self, z, P, W, tag, split=1):
        """z: fp32 AP (P, W) flat view.  Returns (feat, sil):
        feat (P, 8, W) bf16 basis planes (x6 scale folded in weights),
        sil  (P, W) bf16 silu(z).  split>1 chops the free axis so work can
        start before the whole input plane is ready."""
        nc = self.nc
        pool = self.feat if P == 128 else self.sfeat
        spool = self.bfa if P == 128 else self.sfeat
        feat = pool.tile([P, 8, W], BF16, tag=f"feat_{128 if P == 128 else 64}")
        sg = self.scr.tile([P, W], F32, tag=f"sg_{128 if P == 128 else 64}")
        sil = spool.tile([P, W], BF16, tag=f"sil_{128 if P == 128 else 64}")
        S = W // split
        for s in range(split):
            # split>1 requires z shaped (P, split, S); whole-plane otherwise
            zs = z[:, s, :] if split > 1 else z
            nc.scalar.activation(sg[:, ts(s, S)], zs, AF.Sigmoid)
            nc.gpsimd.tensor_mul(sil[:, ts(s, S)], zs, sg[:, ts(s, S)])
            for g in range(8):
                v = self.scr.tile(
                    [P, S], F32, tag=f"v_{128 if P == 128 else 64}",
                    name=f"v{s}_{g}",
                )
                nc.vector._custom_dve(
                    VCLAMP, out=v[:], in0=zs, s0=2.5, s1=float(g) - 5.5,
                    imm2=float(g) - 1.5,
                )
                nc.vector._custom_dve(
                    BUMP3, out=feat[:, g, ts(s, S)], in0=v[:], s0=-4.0
                )
        return feat, sil

    # ---- matmul over features ------------------------------------------- #
    def kan_matmul_512(self, feat, sil, w, out_cb):
        """feat (128,8,1024), sil (128,1024), w (128,36,4,128) bf16 lhsT.
        For each m-tile: psum (128,256) after 36 accumulating matmuls ->
        out_cb(m, psum_ap)."""
        nc = self.nc
        pms = [
            self.psum4.tile([128, NTOK], F32, tag="pmm", name=f"pmm{m}")
            for m in range(4)
        ]
        gorder = [8] + list(range(8))
        for gi, g in enumerate(gorder):
            for k in range(4):
                rhs = sil[:, ts(k, NTOK)] if g == 8 else feat[:, g, ts(k, NTOK)]
                for m in range(4):
                    nc.tensor.matmul(
                        pms[m][:], w[:, g * 4 + k, m, :], rhs,
                        start=(gi == 0 and k == 0), stop=(gi == 8 and k == 3),
                    )
        for m in range(4):
            out_cb(m, pms[m])

    def kan_matmul_512_to_64(self, feat, sil, w):
        """-> psum (64, 256) after 36 matmuls. w (128, 36, 64)."""
        nc = self.nc
        pm = self.psum1.tile([64, NTOK], F32, tag="pk64")
        n = 0
        for g in [8] + list(range(8)):
            for k in range(4):
                rhs = sil[:, ts(k, NTOK)] if g == 8 else feat[:, g, ts(k, NTOK)]
                nc.tensor.matmul(
                    pm[:], w[:, g * 4 + k, :], rhs, start=(n == 0), stop=(n == 35)
                )
                n += 1
        return pm

    def kan_matmul_64_to_512(self, feat, sil, w, out_cb):
        """feat (64,8,256), sil (64,256), w (64,9,4,128)."""
        nc = self.nc
        pms = [
            self.psum4.tile([128, NTOK], F32, tag="pmm", name=f"pmm{m}")
            for m in range(4)
        ]
        gorder = [8] + list(range(8))
        for gi, g in enumerate(gorder):
            rhs = sil[:] if g == 8 else feat[:, g, :]
            for m in range(4):
                nc.tensor.matmul(
                    pms[m][:], w[:, g, m, :], rhs, start=(gi == 0), stop=(gi == 8)
                )
        for m in range(4):
            out_cb(m, pms[m])

    # ---- layernorm ------------------------------------------------------ #
    def stats_from(self, srcs):
        """srcs: list of (b, ap) free-dim slabs covering each batch; emits
        Identity+Square accum passes and returns stats tile (128, n) with
        layout [sum, sumsq] per accum slot plus the slot->batch map."""
        nc = self.nc
        n = len(srcs)
        stats = self.tiny.tile([128, 2 * n], F32, name="stats")
        for j, (b, sl) in enumerate(srcs):
            scr1 = self.scr.tile(list(sl.shape), F32, tag="sqscr", name=f"scr1_{j}")
            nc.scalar.activation(
                scr1[:], sl, AF.Identity, accum_out=stats[:, 2 * j : 2 * j + 1]
            )
            sqr = self.scr.tile(list(sl.shape), F32, tag="sqscr", name=f"sqr_{j}")
            nc.scalar.activation(
                sqr[:], sl, AF.Square, accum_out=stats[:, 2 * j + 1 : 2 * j + 2]
            )
        return stats

    def layernorm(self, xT, zname, lnw=None, lnb=None, stats=None, smap=None,
                  neng=None):
        """xT (128, 4, NTOK) fp32 -> z normalized per batch.  stats: tile
        (128, 2n) of [sum, sumsq] accum slots; smap[j] = batch of slot j
        (slots of one batch are summed)."""
        nc = self.nc
        if stats is None:
            stats = self.stats_from(
                [(b, xT[:, :, ts(b, C4)]) for b in range(BPC)]
            )
            smap = list(range(BPC))
        neng = neng or self.nc.gpsimd
        n2 = stats.shape[1]
        pstat = self.psum.tile([128, 128], F32, tag="ptr", name="pstat")[:, :n2]
        nc.tensor.matmul(pstat[:], self.ones[:], stats[:], start=True, stop=True)
        statsG = self.tiny.tile([128, n2], F32, name="statsG")
        nc.vector.tensor_scalar(
            out=statsG[:], in0=pstat[:], scalar1=INV_CNT, scalar2=None,
            op0=mybir.AluOpType.mult,
        )
        if len(smap) > BPC:
            # fold multiple slots per batch (pairwise into statsF)
            statsF = self.tiny.tile([128, 2 * BPC], F32, name="statsF")
            for b in range(BPC):
                idx = [j for j, bb in enumerate(smap) if bb == b]
                dst = statsF[:, 2 * b : 2 * b + 2]
                neng.tensor_add(
                    dst, statsG[:, 2 * idx[0] : 2 * idx[0] + 2],
                    statsG[:, 2 * idx[1] : 2 * idx[1] + 2],
                )
                for j in idx[2:]:
                    neng.tensor_add(dst, dst, statsG[:, 2 * j : 2 * j + 2])
        else:
            statsF = statsG
        mu = statsF[:, 0 : 2 * BPC : 2]
        e2 = statsF[:, 1 : 2 * BPC : 2]
        var = self.tiny.tile([128, BPC], F32)
        neng.tensor_mul(var[:], mu, mu)
        neng.tensor_sub(var[:], e2, var[:])
        a = self.tiny.tile([128, BPC], F32)
        neng.tensor_scalar_add(a[:], var[:], EPS)
        # y = rsqrt(a) by Newton from y0 = min(1/a, 1) (monotone from below)
        y = self.tiny.tile([128, BPC], F32)
        nc.vector.reciprocal(y[:], a[:])
        neng.tensor_scalar_min(y[:], y[:], 1.0)
        t = self.tiny.tile([128, BPC], F32)
        for _ in range(12):
            neng.tensor_mul(t[:], y[:], y[:])
            neng.tensor_mul(t[:], t[:], a[:])
            neng.tensor_scalar(
                out=t[:], in0=t[:], scalar1=-0.5, scalar2=1.5,
                op0=mybir.AluOpType.mult, op1=mybir.AluOpType.add,
            )
            neng.tensor_mul(y[:], y[:], t[:])
        musc = self.tiny.tile([128, BPC], F32)
        neng.tensor_mul(musc[:], mu, y[:])
        z = self.act.tile([128, 4, NTOK], F32, tag=zname)
        for b in range(BPC):
            nc.vector.tensor_scalar(
                out=z[:, :, ts(b, C4)], in0=xT[:, :, ts(b, C4)],
                scalar1=y[:, b : b + 1], scalar2=musc[:, b : b + 1],
                op0=mybir.AluOpType.mult, op1=mybir.AluOpType.subtract,
            )
        if lnw is not None:
            nc.vector.tensor_mul(z[:], z[:], lnw[:])
        if lnb is not None:
            nc.vector.tensor_add(z[:], z[:], lnb[:])
        return z

    # ---- gcn (folded) ---------------------------------------------------- #
    def gcn(self, tm_bf, wg, bias, bias_sc, yname):
        """tm_bf (128,4,NTOK) bf16; wg (128,4,4,128) bf16; bias (128,4) f32.
        Returns y (128,4,NTOK) f32 = gelu(tm @ Wg + b)."""
        nc = self.nc
        y = self.act.tile([128, 4, NTOK], F32, tag=yname)
        for m in range(4):
            pm = self.psum4.tile([128, NTOK], F32, tag="pmm")
            for k in range(4):
                nc.tensor.matmul(
                    pm[:], wg[:, k, m, :], tm_bf[:, k, :],
                    start=(k == 0), stop=(k == 3),
                )
            hb = self.scr.tile([128, NTOK], F32, tag="hb")
            nc.scalar.activation(hb[:], pm[:], AF.Identity, bias=bias[:, m : m + 1])
            e = self.scr.tile([128, NTOK], F32, tag="erf")
            nc.scalar.activation(
                e[:], pm[:], AF.Erf, bias=bias_sc[:, m : m + 1], scale=ISQ2
            )
            nc.vector._custom_dve(
                GELU_FIN, out=y[:, m, :], in0=hb[:], in1=e[:], s0=0.5
            )
        return y


def _emit(nc, ln_flags):
    """Emit the full per-core kernel.  ln_flags = (use_lnw1, use_lnb1,
    use_lnw2, use_lnb2) -- whether the TM layernorm affine params are
    non-trivial and must be applied."""
    use_lnw1, use_lnb1, use_lnw2, use_lnb2 = ln_flags
    dram = {}

    def din(name, shape, dt=BF16):
        dram[name] = nc.dram_tensor(name, shape, dt, kind="ExternalInput").ap()
        return dram[name]

    x_d = din("x_sh", (BPC, C4, T), F32)
    w_tm1k1 = din("w_tm1k1", (128, 36, 64))
    w_tm1k2 = din("w_tm1k2", (64, 9, 4, 128))
    w_k1 = din("w_k1", (128, 36, 4, 128))
    w_g1 = din("w_g1", (128, 4, 4, 128))
    b_g1 = din("b_g1", (128, 4, 2), F32)        # [:, :, 0]=b, [:, :, 1]=b/sqrt2
    w_tm2k1 = din("w_tm2k1", (128, 36, 64))
    w_tm2k2 = din("w_tm2k2", (64, 9, 4, 128))
    w_g2 = din("w_g2", (128, 4, 4, 128))
    b_g2 = din("b_g2", (128, 4, 2), F32)
    w_k2 = din("w_k2", (128, 36, 4, 128))
    ln1w_d = din("ln1w", (128, 4, NTOK), F32) if use_lnw1 else None
    ln1b_d = din("ln1b", (128, 4, NTOK), F32) if use_lnb1 else None
    ln2w_d = din("ln2w", (128, 4, NTOK), F32) if use_lnw2 else None
    ln2b_d = din("ln2b", (128, 4, NTOK), F32) if use_lnb2 else None
    out_d = nc.dram_tensor("out_sh", (BPC, C4, T), F32, kind="ExternalOutput").ap()

    with tile.TileContext(nc) as tc, ExitStack() as ctx:
        kb = _KB(nc, tc, ctx)
        wpool = ctx.enter_context(tc.tile_pool(name="weights", bufs=1))
        sync = nc.sync

        # ---- weight/input DMA (issue in consumption order) ----
        def wload(ap, shape, tag, dt=BF16):
            t = wpool.tile(list(shape), dt, tag=tag)
            sync.dma_start(t[:], ap)
            return t

        xN = kb.act.tile([C4, BPC, T], F32, tag="nat")
        x_r = x_d.rearrange("b p t -> p b t")
        for k in range(4):
            for b in range(BPC):
                sync.dma_start(
                    xN[:, b, ts(k, 128)], x_r[:, b, ts(k, 128)]
                )
        W_k1 = wload(w_k1, (128, 36, 4, 128), "wk_big")
        W_tm2k1 = wload(w_tm2k1, (128, 36, 64), "wtm2k1")
        W_tm2k2 = wload(w_tm2k2, (64, 9, 4, 128), "wtm2k2")
        W_g2 = wload(w_g2, (128, 4, 4, 128), "wg2")
        B_g2 = wload(b_g2, (128, 4, 2), "bg2", F32)
        W_tm1k1 = wload(w_tm1k1, (128, 36, 64), "wtm1k1")
        W_tm1k2 = wload(w_tm1k2, (64, 9, 4, 128), "wtm1k2")
        W_g1 = wload(w_g1, (128, 4, 4, 128), "wg1")
        B_g1 = wload(b_g1, (128, 4, 2), "bg1", F32)
        W_k2 = wload(w_k2, (128, 36, 4, 128), "wk_big")
        LN1W = wload(ln1w_d, (128, 4, NTOK), "ln1w", F32) if use_lnw1 else None
        LN1B = wload(ln1b_d, (128, 4, NTOK), "ln1b", F32) if use_lnb1 else None
        LN2W = wload(ln2w_d, (128, 4, NTOK), "ln2w", F32) if use_lnw2 else None
        LN2B = wload(ln2b_d, (128, 4, NTOK), "ln2b", F32) if use_lnb2 else None

        # ---- transpose x into T-layout ----
        xT = kb.act.tile([128, 4, NTOK], F32, tag="xT")
        for k in range(4):
            for b in range(BPC):
                pt = kb.psum.tile([128, 128], F32, tag="ptr")
                nc.tensor.transpose(pt[:], xN[:, b, ts(k, 128)], kb.ident[:])
                nc.scalar.copy(xT[:, k, ts(b, 128)], pt[:])

        # ---- TM1 stats from xN (ready before transposes finish) ----
        stats1 = kb.stats_from([(b, xN[:, b, :]) for b in range(BPC)])
        z1 = kb.layernorm(xT, "z", LN1W, LN1B, stats=stats1,
                          smap=list(range(BPC)))
        featX, silX = kb.kan_features(xT[:, :, :], 128, 4 * NTOK, "x", split=4)
        featA, silA = kb.kan_features(z1[:, :, :], 128, 4 * NTOK, "a")

        # ---- k1 matmuls -> cm ----
        cm = kb.act.tile([128, 4, NTOK], F32, tag="cm")
        kb.kan_matmul_512(
            featX, silX, W_k1,
            lambda m, pm: nc.scalar.copy(cm[:, m, :], pm[:]),
        )

        # ---- TM1 kan1 -> z2, kan2 -> tm1 ----
        p1 = kb.kan_matmul_512_to_64(featA, silA, W_tm1k1)
        z2 = kb.act.tile([64, NTOK], F32, tag="z64")
        nc.scalar.copy(z2[:], p1[:])
        featB, silB = kb.kan_features(z2[:, :], 64, NTOK, "b")
        tm1 = kb.bfa.tile([128, 4, NTOK], BF16, tag="tm")
        kb.kan_matmul_64_to_512(
            featB, silB, W_tm1k2,
            lambda m, pm: nc.vector.tensor_add(tm1[:, m, :], xT[:, m, :], pm[:]),
        )

        # ---- TM2 on cm ----
        z3 = kb.layernorm(cm, "z", LN2W, LN2B)
        featC, silC = kb.kan_features(z3[:, :, :], 128, 4 * NTOK, "c")
        p2 = kb.kan_matmul_512_to_64(featC, silC, W_tm2k1)
        z4 = kb.act.tile([64, NTOK], F32, tag="z64")
        nc.scalar.copy(z4[:], p2[:])
        featD, silD = kb.kan_features(z4[:, :], 64, NTOK, "d")
        tm2 = kb.bfa.tile([128, 4, NTOK], BF16, tag="tm")
        kb.kan_matmul_64_to_512(
            featD, silD, W_tm2k2,
            lambda m, pm: nc.vector.tensor_add(tm2[:, m, :], cm[:, m, :], pm[:]),
        )

        # ---- gcn1 (y1) and gcn2 (y2) ----
        y1 = kb.gcn(tm1, W_g1, B_g1[:, :, 0], B_g1[:, :, 1], "y")
        y2 = kb.gcn(tm2, W_g2, B_g2[:, :, 0], B_g2[:, :, 1], "y")

        # ---- k2 on y2, final add, transpose out ----
        featY, silY = kb.kan_features(y2[:, :, :], 128, 4 * NTOK, "y")
        outT = kb.act.tile([128, 4, NTOK], F32, tag="outT")
        kb.kan_matmul_512(
            featY, silY, W_k2,
            lambda m, pm: nc.vector.tensor_add(outT[:, m, :], y1[:, m, :], pm[:]),
        )

        outN = kb.act.tile([C4, BPC, T], F32, tag="nat")
        out_r = out_d.rearrange("b p t -> p b t")
        for m in range(4):
            for b in range(BPC):
                pt = kb.psum.tile([128, 128], F32, tag="ptr")
                nc.tensor.transpose(pt[:], outT[:, m, ts(b, 128)], kb.ident[:])
                nc.scalar.copy(outN[:, b, ts(m, 128)], pt[:])
            sync.dma_start(
                out_r[:, :, ts(m, 128)], outN[:, :, ts(m, 128)]
            )

    return dram


def _build(ln_flags):
    key = ln_flags
    if key in _COMPILED:
        return _COMPILED[key]
    nc = bacc.Bacc("TRN2", target_bir_lowering=False, debug=False)
    _emit(nc, ln_flags)
    nc.compile()
    _COMPILED[key] = nc
    return nc


# --------------------------------------------------------------------------
# host-side weight preparation
# --------------------------------------------------------------------------
def _prep_kan_512(base_w, spline_w):
    """base_w (O,512), spline_w (O,512,8) -> (128, 36, O) or (128,36,4,128)."""
    O = base_w.shape[0]
    w = np.empty((128, 36, O), np.float32)
    for g in range(8):
        for k in range(4):
            # rows p -> channel 128k+p, feature basis g (x 1/6)
            w[:, g * 4 + k, :] = spline_w[:, k * 128 : (k + 1) * 128, g].T / 6.0
    for k in range(4):
        w[:, 32 + k, :] = base_w[:, k * 128 : (k + 1) * 128].T
    w = w.astype(BF)
    if O == 512:
        return np.ascontiguousarray(w.reshape(128, 36, 4, 128))
    return np.ascontiguousarray(w)


def _prep_kan_64(base_w, spline_w):
    """base_w (512,64), spline_w (512,64,8) -> (64, 9, 4, 128)."""
    w = np.empty((64, 9, 4, 128), np.float32)
    for g in range(8):
        for m in range(4):
            w[:, g, m, :] = spline_w[m * 128 : (m + 1) * 128, :, g].T / 6.0
    for m in range(4):
        w[:, 8, m, :] = base_w[m * 128 : (m + 1) * 128, :].T
    return np.ascontiguousarray(w.astype(BF))


def _prep_gcn(gw, gb):
    """gw (512, 1536) -> folded (128,4,4,128) bf16 lhsT; gb -> (128,4,2) f32."""
    Wf = gw[:, :512] + gw[:, 512:1024] + gw[:, 1024:]
    w = np.empty((128, 4, 4, 128), np.float32)
    for k in range(4):
        for m in range(4):
            w[:, k, m, :] = Wf[m * 128 : (m + 1) * 128, k * 128 : (k + 1) * 128].T
    b = np.empty((128, 4, 2), np.float32)
    b[:, :, 0] = gb.reshape(4, 128).T
    b[:, :, 1] = b[:, :, 0] * ISQ2
    return np.ascontiguousarray(w.astype(BF)), np.ascontiguousarray(b)


def _ln_plane(a):
    """ln param (512, 128) -> (128, 4, NTOK) f32 duplicated over batches."""
    p = np.empty((128, 4, NTOK), np.float32)
    for k in range(4):
        for b in range(BPC):
            p[:, k, b * C4 : (b + 1) * C4] = a[k * 128 : (k + 1) * 128, :]
    return np.ascontiguousarray(p)


def kernel(**inputs):
    i = {k: np.asarray(v) for k, v in inputs.items()}
    use_lnw1 = not np.all(i["tm1_ln_w"] == 1.0)
    use_lnb1 = not np.all(i["tm1_ln_b"] == 0.0)
    use_lnw2 = not np.all(i["tm_ln_w"] == 1.0)
    use_lnb2 = not np.all(i["tm_ln_b"] == 0.0)
    ln_flags = (use_lnw1, use_lnb1, use_lnw2, use_lnb2)
    nc = _build(ln_flags)

    w_tm1k1 = _prep_kan_512(i["tm1_k1_base"], i["tm1_k1_spline"])
    w_tm1k2 = _prep_kan_64(i["tm1_k2_base"], i["tm1_k2_spline"])
    w_k1 = _prep_kan_512(i["k1_base"], i["k1_spline"])
    w_g1, b_g1 = _prep_gcn(i["g1_w"], i["g1_b"])
    w_tm2k1 = _prep_kan_512(i["tm_k1_base"], i["tm_k1_spline"])
    w_tm2k2 = _prep_kan_64(i["tm_k2_base"], i["tm_k2_spline"])
    w_g2, b_g2 = _prep_gcn(i["g2_w"], i["g2_b"])
    w_k2 = _prep_kan_512(i["k2_base"], i["k2_spline"])

    shared = dict(
        w_tm1k1=w_tm1k1, w_tm1k2=w_tm1k2, w_k1=w_k1, w_g1=w_g1, b_g1=b_g1,
        w_tm2k1=w_tm2k1, w_tm2k2=w_tm2k2, w_g2=w_g2, b_g2=b_g2, w_k2=w_k2,
    )
    if use_lnw1:
        shared["ln1w"] = _ln_plane(i["tm1_ln_w"])
    if use_lnb1:
        shared["ln1b"] = _ln_plane(i["tm1_ln_b"])
    if use_lnw2:
        shared["ln2w"] = _ln_plane(i["tm_ln_w"])
    if use_lnb2:
        shared["ln2b"] = _ln_plane(i["tm_ln_b"])
    x = np.ascontiguousarray(i["x"], np.float32)
    in_maps = [
        {"x_sh": x[c * BPC : (c + 1) * BPC], **shared} for c in range(NCORES)
    ]
    res = run_bass_kernel_spmd(nc, in_maps, core_ids=list(range(NCORES)))
    out = np.empty((B, C4, T), np.float32)
    for c in range(NCORES):
        out[c * BPC : (c + 1) * BPC] = res.results[c]["out_sh"]
    return out


# revision 26
# speedup vs baseline: 1.1921x; 1.0087x over previous
"""Trainium2 Bass kernel for nn_Mixer2dTriUKAN_66417374265858.

Mathematical simplification: in gcn_spatial the adjacency enters only as
s = sum(softmax(P), axis=-1) == 1, so the entire FFT/prob_distance/softmax
branch cancels and gcn_spatial(x, a, w, b) == gelu(x @ (w1+w2+w3).T + b)
where w = [w1|w2|w3] split along the 3T axis.  (Verified: rel err ~9e-7.)

What remains per batch (B=16, C4=128 tokens, T=D=512):
  tm1 = TM(x)   = x + kan64->512(kan512->64(LN(x)))
  y1  = gelu(tm1 @ W1f.T + b1)
  cm  = kan512->512(x)
  tm2 = TM(cm)
  y2  = gelu(tm2 @ W2f.T + b2)
  out = y1 + kan512->512(y2)

kan(x) = silu(x) @ Wb.T + bspl(x) (.) Ws, with the 8 cubic B-spline bases
computed per element as basis_i(x) = (v^3 - 4*relu(v-1)^3)/6 where
v = relu(min(u-i, (i+4)-u)), u = 2.5x + 5.5 -- two fused custom DVE ops
(KAN_VCLAMP 5 ALU stages, KAN_BUMP3 8 stages) per basis.

Sharding: data-parallel over batch, 2 batches per core on 8 cores, weights
replicated.  All activations live in "transposed" layout (feature dim on
partitions, 256 = 2x128 tokens on the free axis); matmuls contract over the
partition axis with bf16 inputs and fp32 PSUM accumulation.
"""
from contextlib import ExitStack

import numpy as np
import ml_dtypes

import concourse.bacc as bacc
import concourse.bass as bass
import concourse.mybir as mybir
import concourse.tile as tile
from concourse.bass import ts
from concourse.bass_utils import run_bass_kernel_spmd
from concourse.masks import make_identity

import concourse.dve_ops as dve_ops
from concourse.dve_ops import DveOp
from concourse.dve_spec import Spec, Src0, Src1, C0, C1, C2, One, relu, sq, minn, lower
from concourse.dve_uop import DveOpSpec

BF = ml_dtypes.bfloat16
F32 = mybir.dt.float32
BF16 = mybir.dt.bfloat16
AF = mybir.ActivationFunctionType

B, C4, T = 16, 128, 512
NCORES = 8
BPC = B // NCORES          # batches per core
NTOK = BPC * C4            # 256 tokens on the free axis
INV_CNT = 1.0 / (C4 * T)   # layernorm element count per batch
EPS = 1e-5
ISQ2 = float(1.0 / np.sqrt(2.0))

_COMPILED = {}             # cache: key -> (nc, input_names)


# --------------------------------------------------------------------------
# custom DVE ops (registered at import, idempotent)
# --------------------------------------------------------------------------
def _mk_op(name, spec, subdim=False):
    shas = {}
    for ver in ("v3", "v4"):
        try:
            s = DveOpSpec(name=name, opcode=0, uops=lower(spec, ver=ver))
            shas[ver] = s.sha(ver)
        except Exception:
            pass
    return DveOp(name, spec, subdim=subdim, uops_sha=shas)


def _register_ops():
    have = {op.name for op in dve_ops.OPS}
    out = {}
    m = Src0 * C0
    _r = relu(Src0 - One)   # shared subexpression: computed once
    specs = {
        # relu(min(x*s0 - s1, imm2 - x*s0))
        "KAN_VCLAMP": Spec(
            body=relu(minn(m - C1, C2 - m)),
            reference=lambda in0, in1, s0, s1, imm2: np.maximum(
                np.minimum(in0 * s0 - s1, imm2 - in0 * s0), 0.0
            ),
        ),
        # v^3 + s0*relu(v-1)^3   (s0=-4)
        "KAN_BUMP3": Spec(
            body=sq(Src0) * Src0 + (sq(_r) * _r) * C0,
            reference=lambda in0, in1, s0, s1, imm2: in0**3
            + s0 * np.maximum(in0 - 1.0, 0.0) ** 3,
        ),
        # (in0*s0) * (in1 + 1)    -- gelu finish: 0.5*h*(1+erf(h/sqrt2))
        "GELU_FIN": Spec(
            body=(Src0 * C0) * (Src1 + One),
            reference=lambda in0, in1, s0, s1, imm2: (in0 * s0) * (in1 + 1.0),
        ),
    }
    for name, spec in specs.items():
        if name in have:
            out[name] = next(op for op in dve_ops.OPS if op.name == name)
            continue
        op = _mk_op(name, spec)
        dve_ops.OPS.append(op)
        dve_ops._SUB_OPCODE_FOR_NAME[name] = (
            dve_ops._CUSTOM_DVE_ROW_BASE + len(dve_ops.OPS) - 1
        )
        dve_ops.CUSTOM_DVE_SPECS[name] = spec
        out[name] = op
    return out


_OPS = _register_ops()
VCLAMP = _OPS["KAN_VCLAMP"]
BUMP3 = _OPS["KAN_BUMP3"]
GELU_FIN = _OPS["GELU_FIN"]


# --------------------------------------------------------------------------
# kernel builder
# --------------------------------------------------------------------------
class _KB:
    """Emission helper holding nc/tc/pools."""

    def __init__(self, nc, tc, ctx):
        self.nc = nc
        self.tc = tc
        p = lambda **kw: ctx.enter_context(tc.tile_pool(**kw))
        self.singles = p(name="singles", bufs=1)
        self.act = p(name="act", bufs=2)        # activation planes (z/cm/y/...)
        self.feat = p(name="feat", bufs=2)      # big bf16 feature buffers
        self.sfeat = p(name="sfeat", bufs=2)    # small (64p) feature buffers
        self.scr = p(name="scr", bufs=2)        # fp32 scratch (v planes, squares)
        self.tiny = p(name="tiny", bufs=8)      # stats vectors
        self.bfa = p(name="bfa", bufs=2)        # bf16 activation planes
        self.psum4 = p(name="psum4", bufs=5, space="PSUM")
        self.psum = p(name="psum", bufs=2, space="PSUM")
        self.psum1 = p(name="psum1", bufs=1, space="PSUM")

        self.ident = self.singles.tile([128, 128], F32)
        make_identity(nc, self.ident[:])
        self.ones = self.singles.tile([128, 128], F32)
        nc.gpsimd.memset(self.ones[:], 1.0)

    # ---- b-spline + silu feature construction --------------------------- #
    def kan_features(self, z, P, W, tag, split=1):
        """z: fp32 AP (P, W) flat view.  Returns (feat, sil):
        feat (P, 8, W) bf16 basis planes (x6 scale folded in weights),
        sil  (P, W) bf16 silu(z).  split>1 chops the free axis so work can
        start before the whole input plane is ready."""
        nc = self.nc
        pool = self.feat if P == 128 else self.sfeat
        spool = self.bfa if P == 128 else self.sfeat
        feat = pool.tile([P, 8, W], BF16, tag=f"feat_{128 if P == 128 else 64}")
        sg = self.scr.tile([P, W], F32, tag=f"sg_{128 if P == 128 else 64}")
        sil = spool.tile([P, W], BF16, tag=f"sil_{128 if P == 128 else 64}")
        S = W // split
        for s in range(split):
            # split>1 requires z shaped (P, split, S); whole-plane otherwise
            zs = z[:, s, :] if split > 1 else z
            nc.scalar.activation(sg[:, ts(s, S)], zs, AF.Sigmoid)
            nc.gpsimd.tensor_mul(sil[:, ts(s, S)], zs, sg[:, ts(s, S)])
            for g in range(8):
                v = self.scr.tile(
                    [P, S], F32, tag=f"v_{128 if P == 128 else 64}",
                    name=f"v{s}_{g}",
                )
                nc.vector._custom_dve(
                    VCLAMP, out=v[:], in0=zs, s0=2.5, s1=float(g) - 5.5,
                    imm2=float(g) - 1.5,
                )
                nc.vector._custom_dve(
                    BUMP3, out=feat[:, g, ts(s, S)], in0=v[:], s0=-4.0
                )
        return feat, sil

    # ---- matmul over features ------------------------------------------- #
    def kan_matmul_512(self, feat, sil, w, out_cb):
        """feat (128,8,1024), sil (128,1024), w (128,36,4,128) bf16 lhsT.
        For each m-tile: psum (128,256) after 36 accumulating matmuls ->
        out_cb(m, psum_ap)."""
        nc = self.nc
        pms = [
            self.psum4.tile([128, NTOK], F32, tag="pmm", name=f"pmm{m}")
            for m in range(4)
        ]
        gorder = [8] + list(range(8))
        for gi, g in enumerate(gorder):
            for k in range(4):
                rhs = sil[:, ts(k, NTOK)] if g == 8 else feat[:, g, ts(k, NTOK)]
                for m in range(4):
                    nc.tensor.matmul(
                        pms[m][:], w[:, g * 4 + k, m, :], rhs,
                        start=(gi == 0 and k == 0), stop=(gi == 8 and k == 3),
                    )
        for m in range(4):
            out_cb(m, pms[m])

    def kan_matmul_512_to_64(self, feat, sil, w):
        """-> psum (64, 256) after 36 matmuls. w (128, 36, 64)."""
        nc = self.nc
        pm = self.psum1.tile([64, NTOK], F32, tag="pk64")
        n = 0
        for g in [8] + list(range(8)):
            for k in range(4):
                rhs = sil[:, ts(k, NTOK)] if g == 8 else feat[:, g, ts(k, NTOK)]
                nc.tensor.matmul(
                    pm[:], w[:, g * 4 + k, :], rhs, start=(n == 0), stop=(n == 35)
                )
                n += 1
        return pm

    def kan_matmul_64_to_512(self, feat, sil, w, out_cb):
        """feat (64,8,256), sil (64,256), w (64,9,4,128)."""
        nc = self.nc
        pms = [
            self.psum4.tile([128, NTOK], F32, tag="pmm", name=f"pmm{m}")
            for m in range(4)
        ]
        gorder = [8] + list(range(8))
        for gi, g in enumerate(gorder):
            rhs = sil[:] if g == 8 else feat[:, g, :]
            for m in range(4):
                nc.tensor.matmul(
                    pms[m][:], w[:, g, m, :], rhs, start=(gi == 0), stop=(gi == 8)
                )
        for m in range(4):
            out_cb(m, pms[m])

    # ---- layernorm ------------------------------------------------------ #
    def stats_from(self, srcs):
        """srcs: list of (b, ap) free-dim slabs covering each batch; emits
        Identity+Square accum passes and returns stats tile (128, n) with
        layout [sum, sumsq] per accum slot plus the slot->batch map."""
        nc = self.nc
        n = len(srcs)
        stats = self.tiny.tile([128, 2 * n], F32, name="stats")
        for j, (b, sl) in enumerate(srcs):
            scr1 = self.scr.tile(list(sl.shape), F32, tag="sqscr", name=f"scr1_{j}")
            nc.scalar.activation(
                scr1[:], sl, AF.Identity, accum_out=stats[:, 2 * j : 2 * j + 1]
            )
            sqr = self.scr.tile(list(sl.shape), F32, tag="sqscr", name=f"sqr_{j}")
            nc.scalar.activation(
                sqr[:], sl, AF.Square, accum_out=stats[:, 2 * j + 1 : 2 * j + 2]
            )
        return stats

    def layernorm(self, xT, zname, lnw=None, lnb=None, stats=None, smap=None,
                  neng=None):
        """xT (128, 4, NTOK) fp32 -> z normalized per batch.  stats: tile
        (128, 2n) of [sum, sumsq] accum slots; smap[j] = batch of slot j
        (slots of one batch are summed)."""
        nc = self.nc
        if stats is None:
            stats = self.stats_from(
                [(b, xT[:, :, ts(b, C4)]) for b in range(BPC)]
            )
            smap = list(range(BPC))
        neng = neng or self.nc.gpsimd
        n2 = stats.shape[1]
        pstat = self.psum.tile([128, 128], F32, tag="ptr", name="pstat")[:, :n2]
        nc.tensor.matmul(pstat[:], self.ones[:], stats[:], start=True, stop=True)
        statsG = self.tiny.tile([128, n2], F32, name="statsG")
        nc.vector.tensor_scalar(
            out=statsG[:], in0=pstat[:], scalar1=INV_CNT, scalar2=None,
            op0=mybir.AluOpType.mult,
        )
        if len(smap) > BPC:
            # fold multiple slots per batch (pairwise into statsF)
            statsF = self.tiny.tile([128, 2 * BPC], F32, name="statsF")
            for b in range(BPC):
                idx = [j for j, bb in enumerate(smap) if bb == b]
                dst = statsF[:, 2 * b : 2 * b + 2]
                neng.tensor_add(
                    dst, statsG[:, 2 * idx[0] : 2 * idx[0] + 2],
                    statsG[:, 2 * idx[1] : 2 * idx[1] + 2],
                )
                for j in idx[2:]:
                    neng.tensor_add(dst, dst, statsG[:, 2 * j : 2 * j + 2])
        else:
            statsF = statsG
        mu = statsF[:, 0 : 2 * BPC : 2]
        e2 = statsF[:, 1 : 2 * BPC : 2]
        var = self.tiny.tile([128, BPC], F32)
        neng.tensor_mul(var[:], mu, mu)
        neng.tensor_sub(var[:], e2, var[:])
        a = self.tiny.tile([128, BPC], F32)
        neng.tensor_scalar_add(a[:], var[:], EPS)
        # y = rsqrt(a) by Newton from y0 = min(1/a, 1) (monotone from below)
        y = self.tiny.tile([128, BPC], F32)
        nc.vector.reciprocal(y[:], a[:])
        neng.tensor_scalar_min(y[:], y[:], 1.0)
        t = self.tiny.tile([128, BPC], F32)
        for _ in range(12):
            neng.tensor_mul(t[:], y[:], y[:])
            neng.tensor_mul(t[:], t[:], a[:])
            neng.tensor_scalar(
                out=t[:], in0=t[:], scalar1=-0.5, scalar2=1.5,
                op0=mybir.AluOpType.mult, op1=mybir.AluOpType.add,
            )
            neng.tensor_mul(y[:], y[:], t[:])
        musc = self.tiny.tile([128, BPC], F32)
        neng.tensor_mul(musc[:], mu, y[:])
        z = self.act.tile([128, 4, NTOK], F32, tag=zname)
        for b in range(BPC):
            nc.vector.tensor_scalar(
                out=z[:, :, ts(b, C4)], in0=xT[:, :, ts(b, C4)],
                scalar1=y[:, b : b + 1], scalar2=musc[:, b : b + 1],
                op0=mybir.AluOpType.mult, op1=mybir.AluOpType.subtract,
            )
        if lnw is not None:
            nc.vector.tensor_mul(z[:], z[:], lnw[:])
        if lnb is not None:
            nc.vector.tensor_add(z[:], z[:], lnb[:])
        return z

    # ---- gcn (folded) ---------------------------------------------------- #
    def gcn(self, tm_bf, wg, bias, bias_sc, yname):
        """tm_bf (128,4,NTOK) bf16; wg (128,4,4,128) bf16; bias (128,4) f32.
        Returns y (128,4,NTOK) f32 = gelu(tm @ Wg + b)."""
        nc = self.nc
        y = self.act.tile([128, 4, NTOK], F32, tag=yname)
        for m in range(4):
            pm = self.psum4.tile([128, NTOK], F32, tag="pmm")
            for k in range(4):
                nc.tensor.matmul(
                    pm[:], wg[:, k, m, :], tm_bf[:, k, :],
                    start=(k == 0), stop=(k == 3),
                )
            hb = self.scr.tile([128, NTOK], F32, tag="hb")
            nc.scalar.activation(hb[:], pm[:], AF.Identity, bias=bias[:, m : m + 1])
            e = self.scr.tile([128, NTOK], F32, tag="erf")
            nc.scalar.activation(
                e[:], pm[:], AF.Erf, bias=bias_sc[:, m : m + 1], scale=ISQ2
            )
            nc.vector._custom_dve(
                GELU_FIN, out=y[:, m, :], in0=hb[:], in1=e[:], s0=0.5
            )
        return y


def _emit(nc, ln_flags):
    """Emit the full per-core kernel.  ln_flags = (use_lnw1, use_lnb1,
    use_lnw2, use_lnb2) -- whether the TM layernorm affine params are
    non-trivial and must be applied."""
    use_lnw1, use_lnb1, use_lnw2, use_lnb2 = ln_flags
    dram = {}

    def din(name, shape, dt=BF16):
        dram[name] = nc.dram_tensor(name, shape, dt, kind="ExternalInput").ap()
        return dram[name]

    x_d = din("x_sh", (BPC, C4, T), F32)
    w_tm1k1 = din("w_tm1k1", (128, 36, 64))
    w_tm1k2 = din("w_tm1k2", (64, 9, 4, 128))
    w_k1 = din("w_k1", (128, 36, 4, 128))
    w_g1 = din("w_g1", (128, 4, 4, 128))
    b_g1 = din("b_g1", (128, 4, 2), F32)        # [:, :, 0]=b, [:, :, 1]=b/sqrt2
    w_tm2k1 = din("w_tm2k1", (128, 36, 64))
    w_tm2k2 = din("w_tm2k2", (64, 9, 4, 128))
    w_g2 = din("w_g2", (128, 4, 4, 128))
    b_g2 = din("b_g2", (128, 4, 2), F32)
    w_k2 = din("w_k2", (128, 36, 4, 128))
    ln1w_d = din("ln1w", (128, 4, NTOK), F32) if use_lnw1 else None
    ln1b_d = din("ln1b", (128, 4, NTOK), F32) if use_lnb1 else None
    ln2w_d = din("ln2w", (128, 4, NTOK), F32) if use_lnw2 else None
    ln2b_d = din("ln2b", (128, 4, NTOK), F32) if use_lnb2 else None
    out_d = nc.dram_tensor("out_sh", (BPC, C4, T), F32, kind="ExternalOutput").ap()

    with tile.TileContext(nc) as tc, ExitStack() as ctx:
        kb = _KB(nc, tc, ctx)
        wpool = ctx.enter_context(tc.tile_pool(name="weights", bufs=1))
        sync = nc.sync

        # ---- weight/input DMA (issue in consumption order) ----
        def wload(ap, shape, tag, dt=BF16):
            t = wpool.tile(list(shape), dt, tag=tag)
            sync.dma_start(t[:], ap)
            return t

        xN = kb.act.tile([C4, BPC, T], F32, tag="nat")
        x_r = x_d.rearrange("b p t -> p b t")
        for k in range(4):
            for b in range(BPC):
                sync.dma_start(
                    xN[:, b, ts(k, 128)], x_r[:, b, ts(k, 128)]
                )
        W_k1 = wload(w_k1, (128, 36, 4, 128), "wk_big")
        W_tm2k1 = wload(w_tm2k1, (128, 36, 64), "wtm2k1")
        W_tm2k2 = wload(w_tm2k2, (64, 9, 4, 128), "wtm2k2")
        W_g2 = wload(w_g2, (128, 4, 4, 128), "wg2")
        B_g2 = wload(b_g2, (128, 4, 2), "bg2", F32)
        W_tm1k1 = wload(w_tm1k1, (128, 36, 64), "wtm1k1")
        W_tm1k2 = wload(w_tm1k2, (64, 9, 4, 128), "wtm1k2")
        W_g1 = wload(w_g1, (128, 4, 4, 128), "wg1")
        B_g1 = wload(b_g1, (128, 4, 2), "bg1", F32)
        W_k2 = wload(w_k2, (128, 36, 4, 128), "wk_big")
        LN1W = wload(ln1w_d, (128, 4, NTOK), "ln1w", F32) if use_lnw1 else None
        LN1B = wload(ln1b_d, (128, 4, NTOK), "ln1b", F32) if use_lnb1 else None
        LN2W = wload(ln2w_d, (128, 4, NTOK), "ln2w", F32) if use_lnw2 else None
        LN2B = wload(ln2b_d, (128, 4, NTOK), "ln2b", F32) if use_lnb2 else None

        # ---- transpose x into T-layout ----
        xT = kb.act.tile([128, 4, NTOK], F32, tag="xT")
        for k in range(4):
            for b in range(BPC):
                pt = kb.psum.tile([128, 128], F32, tag="ptr")
                nc.tensor.transpose(pt[:], xN[:, b, ts(k, 128)], kb.ident[:])
                nc.scalar.copy(xT[:, k, ts(b, 128)], pt[:])

        # ---- TM1 stats from xN (ready before transposes finish) ----
        stats1 = kb.stats_from([(b, xN[:, b, :]) for b in range(BPC)])
        z1 = kb.layernorm(xT, "z", LN1W, LN1B, stats=stats1,
                          smap=list(range(BPC)))
        featX, silX = kb.kan_features(xT[:, :, :], 128, 4 * NTOK, "x", split=4)

        # ---- k1 matmuls -> cm (critical chain head) ----
        cm = kb.act.tile([128, 4, NTOK], F32, tag="cm")
        kb.kan_matmul_512(
            featX, silX, W_k1,
            lambda m, pm: nc.scalar.copy(cm[:, m, :], pm[:]),
        )

        # ---- TM1 kan chain + gcn1 (PE work emitted before the blocked
        #      tm2 matmuls so the in-order PE stream isn't inverted) ----
        featA, silA = kb.kan_features(z1[:, :, :], 128, 4 * NTOK, "a")
        p1 = kb.kan_matmul_512_to_64(featA, silA, W_tm1k1)
        z2 = kb.act.tile([64, NTOK], F32, tag="z64")
        nc.scalar.copy(z2[:], p1[:])
        featB, silB = kb.kan_features(z2[:, :], 64, NTOK, "b")
        tm1 = kb.bfa.tile([128, 4, NTOK], BF16, tag="tm")
        kb.kan_matmul_64_to_512(
            featB, silB, W_tm1k2,
            lambda m, pm: nc.vector.tensor_add(tm1[:, m, :], xT[:, m, :], pm[:]),
        )
        y1 = kb.gcn(tm1, W_g1, B_g1[:, :, 0], B_g1[:, :, 1], "y")

        # ---- TM2 on cm ----
        z3 = kb.layernorm(cm, "z", LN2W, LN2B)
        featC, silC = kb.kan_features(z3[:, :, :], 128, 4 * NTOK, "c")
        p2 = kb.kan_matmul_512_to_64(featC, silC, W_tm2k1)
        z4 = kb.act.tile([64, NTOK], F32, tag="z64")
        nc.scalar.copy(z4[:], p2[:])
        featD, silD = kb.kan_features(z4[:, :], 64, NTOK, "d")
        tm2 = kb.bfa.tile([128, 4, NTOK], BF16, tag="tm")
        kb.kan_matmul_64_to_512(
            featD, silD, W_tm2k2,
            lambda m, pm: nc.vector.tensor_add(tm2[:, m, :], cm[:, m, :], pm[:]),
        )
        y2 = kb.gcn(tm2, W_g2, B_g2[:, :, 0], B_g2[:, :, 1], "y")

        # ---- k2 on y2, final add, transpose out ----
        featY, silY = kb.kan_features(y2[:, :, :], 128, 4 * NTOK, "y")
        outT = kb.act.tile([128, 4, NTOK], F32, tag="outT")
        kb.kan_matmul_512(
            featY, silY, W_k2,
            lambda m, pm: nc.vector.tensor_add(outT[:, m, :], y1[:, m, :], pm[:]),
        )

        outN = kb.act.tile([C4, BPC, T], F32, tag="nat")
        out_r = out_d.rearrange("b p t -> p b t")
        for m in range(4):
            for b in range(BPC):
                pt = kb.psum.tile([128, 128], F32, tag="ptr")
                nc.tensor.transpose(pt[:], outT[:, m, ts(b, 128)], kb.ident[:])
                nc.scalar.copy(outN[:, b, ts(m, 128)], pt[:])
            sync.dma_start(
                out_r[:, :, ts(m, 128)], outN[:, :, ts(m, 128)]
            )

    return dram


def _build(ln_flags):
    key = ln_flags
    if key in _COMPILED:
        return _COMPILED[key]
    nc = bacc.Bacc("TRN2", target_bir_lowering=False, debug=False)
    _emit(nc, ln_flags)
    nc.compile()
    _COMPILED[key] = nc
    return nc


# --------------------------------------------------------------------------
# host-side weight preparation
# --------------------------------------------------------------------------
def _prep_kan_512(base_w, spline_w):
    """base_w (O,512), spline_w (O,512,8) -> (128, 36, O) or (128,36,4,128)."""
    O = base_w.shape[0]
    w = np.empty((128, 36, O), np.float32)
    for g in range(8):
        for k in range(4):
            # rows p -> channel 128k+p, feature basis g (x 1/6)
            w[:, g * 4 + k, :] = spline_w[:, k * 128 : (k + 1) * 128, g].T / 6.0
    for k in range(4):
        w[:, 32 + k, :] = base_w[:, k * 128 : (k + 1) * 128].T
    w = w.astype(BF)
    if O == 512:
        return np.ascontiguousarray(w.reshape(128, 36, 4, 128))
    return np.ascontiguousarray(w)


def _prep_kan_64(base_w, spline_w):
    """base_w (512,64), spline_w (512,64,8) -> (64, 9, 4, 128)."""
    w = np.empty((64, 9, 4, 128), np.float32)
    for g in range(8):
        for m in range(4):
            w[:, g, m, :] = spline_w[m * 128 : (m + 1) * 128, :, g].T / 6.0
    for m in range(4):
        w[:, 8, m, :] = base_w[m * 128 : (m + 1) * 128, :].T
    return np.ascontiguousarray(w.astype(BF))


def _prep_gcn(gw, gb):
    """gw (512, 1536) -> folded (128,4,4,128) bf16 lhsT; gb -> (128,4,2) f32."""
    Wf = gw[:, :512] + gw[:, 512:1024] + gw[:, 1024:]
    w = np.empty((128, 4, 4, 128), np.float32)
    for k in range(4):
        for m in range(4):
            w[:, k, m, :] = Wf[m * 128 : (m + 1) * 128, k * 128 : (k + 1) * 128].T
    b = np.empty((128, 4, 2), np.float32)
    b[:, :, 0] = gb.reshape(4, 128).T
    b[:, :, 1] = b[:, :, 0] * ISQ2
    return np.ascontiguousarray(w.astype(BF)), np.ascontiguousarray(b)


def _ln_plane(a):
    """ln param (512, 128) -> (128, 4, NTOK) f32 duplicated over batches."""
    p = np.empty((128, 4, NTOK), np.float32)
    for k in range(4):
        for b in range(BPC):
            p[:, k, b * C4 : (b + 1) * C4] = a[k * 128 : (k + 1) * 128, :]
    return np.ascontiguousarray(p)


def kernel(**inputs):
    i = {k: np.asarray(v) for k, v in inputs.items()}
    use_lnw1 = not np.all(i["tm1_ln_w"] == 1.0)
    use_lnb1 = not np.all(i["tm1_ln_b"] == 0.0)
    use_lnw2 = not np.all(i["tm_ln_w"] == 1.0)
    use_lnb2 = not np.all(i["tm_ln_b"] == 0.0)
    ln_flags = (use_lnw1, use_lnb1, use_lnw2, use_lnb2)
    nc = _build(ln_flags)

    w_tm1k1 = _prep_kan_512(i["tm1_k1_base"], i["tm1_k1_spline"])
    w_tm1k2 = _prep_kan_64(i["tm1_k2_base"], i["tm1_k2_spline"])
    w_k1 = _prep_kan_512(i["k1_base"], i["k1_spline"])
    w_g1, b_g1 = _prep_gcn(i["g1_w"], i["g1_b"])
    w_tm2k1 = _prep_kan_512(i["tm_k1_base"], i["tm_k1_spline"])
    w_tm2k2 = _prep_kan_64(i["tm_k2_base"], i["tm_k2_spline"])
    w_g2, b_g2 = _prep_gcn(i["g2_w"], i["g2_b"])
    w_k2 = _prep_kan_512(i["k2_base"], i["k2_spline"])

    shared = dict(
        w_tm1k1=w_tm1k1, w_tm1k2=w_tm1k2, w_k1=w_k1, w_g1=w_g1, b_g1=b_g1,
        w_tm2k1=w_tm2k1, w_tm2k2=w_tm2k2, w_g2=w_g2, b_g2=b_g2, w_k2=w_k2,
    )
    if use_lnw1:
        shared["ln1w"] = _ln_plane(i["tm1_ln_w"])
    if use_lnb1:
        shared["ln1b"] = _ln_plane(i["tm1_ln_b"])
    if use_lnw2:
        shared["ln2w"] = _ln_plane(i["tm_ln_w"])
    if use_lnb2:
        shared["ln2b"] = _ln_plane(i["tm_ln_b"])
    x = np.ascontiguousarray(i["x"], np.float32)
    in_maps = [
        {"x_sh": x[c * BPC : (c + 1) * BPC], **shared} for c in range(NCORES)
    ]
    res = run_bass_kernel_spmd(nc, in_maps, core_ids=list(range(NCORES)))
    out = np.empty((B, C4, T), np.float32)
    for c in range(NCORES):
        out[c * BPC : (c + 1) * BPC] = res.results[c]["out_sh"]
    return out


# revision 43
# speedup vs baseline: 1.2338x; 1.0350x over previous
"""Trainium2 Bass kernel for nn_Mixer2dTriUKAN_66417374265858.

Mathematical simplification: in gcn_spatial the adjacency enters only as
s = sum(softmax(P), axis=-1) == 1, so the entire FFT/prob_distance/softmax
branch cancels and gcn_spatial(x, a, w, b) == gelu(x @ (w1+w2+w3).T + b)
where w = [w1|w2|w3] split along the 3T axis.  (Verified: rel err ~9e-7.)

What remains per batch (B=16, C4=128 tokens, T=D=512):
  tm1 = TM(x)   = x + kan64->512(kan512->64(LN(x)))
  y1  = gelu(tm1 @ W1f.T + b1)
  cm  = kan512->512(x)
  tm2 = TM(cm)
  y2  = gelu(tm2 @ W2f.T + b2)
  out = y1 + kan512->512(y2)

kan(x) = silu(x) @ Wb.T + bspl(x) (.) Ws, with the 8 cubic B-spline bases
computed per element as basis_i(x) = (v^3 - 4*relu(v-1)^3)/6 where
v = relu(min(u-i, (i+4)-u)), u = 2.5x + 5.5 -- two fused custom DVE ops
(KAN_VCLAMP 5 ALU stages, KAN_BUMP3 8 stages) per basis.

Sharding: data-parallel over batch, 2 batches per core on 8 cores, weights
replicated.  All activations live in "transposed" layout (feature dim on
partitions, 256 = 2x128 tokens on the free axis); matmuls contract over the
partition axis with bf16 inputs and fp32 PSUM accumulation.
"""
from contextlib import ExitStack

import numpy as np
import ml_dtypes

import concourse.bacc as bacc
import concourse.bass as bass
import concourse.mybir as mybir
import concourse.tile as tile
from concourse.bass import ts
from concourse.bass_utils import run_bass_kernel_spmd
from concourse.masks import make_identity

import concourse.dve_ops as dve_ops
from concourse.dve_ops import DveOp
from concourse.dve_spec import Spec, Src0, Src1, C0, C1, C2, One, relu, sq, minn, lower
from concourse.dve_uop import DveOpSpec

BF = ml_dtypes.bfloat16
F32 = mybir.dt.float32
BF16 = mybir.dt.bfloat16
AF = mybir.ActivationFunctionType

B, C4, T = 16, 128, 512
NCORES = 8
BPC = B // NCORES          # batches per core
NTOK = BPC * C4            # 256 tokens on the free axis
INV_CNT = 1.0 / (C4 * T)   # layernorm element count per batch
EPS = 1e-5
ISQ2 = float(1.0 / np.sqrt(2.0))

_COMPILED = {}             # cache: key -> (nc, input_names)


# --------------------------------------------------------------------------
# custom DVE ops (registered at import, idempotent)
# --------------------------------------------------------------------------
def _mk_op(name, spec, subdim=False):
    shas = {}
    for ver in ("v3", "v4"):
        try:
            s = DveOpSpec(name=name, opcode=0, uops=lower(spec, ver=ver))
            shas[ver] = s.sha(ver)
        except Exception:
            pass
    return DveOp(name, spec, subdim=subdim, uops_sha=shas)


def _register_ops():
    have = {op.name for op in dve_ops.OPS}
    out = {}
    m = Src0 * C0
    _r = relu(Src0 - One)   # shared subexpression: computed once
    specs = {
        # relu(min(x*s0 - s1, imm2 - x*s0))
        "KAN_VCLAMP": Spec(
            body=relu(minn(m - C1, C2 - m)),
            reference=lambda in0, in1, s0, s1, imm2: np.maximum(
                np.minimum(in0 * s0 - s1, imm2 - in0 * s0), 0.0
            ),
        ),
        # v^3 + s0*relu(v-1)^3   (s0=-4)
        "KAN_BUMP3": Spec(
            body=sq(Src0) * Src0 + (sq(_r) * _r) * C0,
            reference=lambda in0, in1, s0, s1, imm2: in0**3
            + s0 * np.maximum(in0 - 1.0, 0.0) ** 3,
        ),
        # (in0*s0) * (in1 + 1)    -- gelu finish: 0.5*h*(1+erf(h/sqrt2))
        "GELU_FIN": Spec(
            body=(Src0 * C0) * (Src1 + One),
            reference=lambda in0, in1, s0, s1, imm2: (in0 * s0) * (in1 + 1.0),
        ),
    }
    for name, spec in specs.items():
        if name in have:
            out[name] = next(op for op in dve_ops.OPS if op.name == name)
            continue
        op = _mk_op(name, spec)
        dve_ops.OPS.append(op)
        dve_ops._SUB_OPCODE_FOR_NAME[name] = (
            dve_ops._CUSTOM_DVE_ROW_BASE + len(dve_ops.OPS) - 1
        )
        dve_ops.CUSTOM_DVE_SPECS[name] = spec
        out[name] = op
    return out


_OPS = _register_ops()
VCLAMP = _OPS["KAN_VCLAMP"]
BUMP3 = _OPS["KAN_BUMP3"]
GELU_FIN = _OPS["GELU_FIN"]


# --------------------------------------------------------------------------
# kernel builder
# --------------------------------------------------------------------------
class _KB:
    """Emission helper holding nc/tc/pools."""

    def __init__(self, nc, tc, ctx):
        self.nc = nc
        self.tc = tc
        p = lambda **kw: ctx.enter_context(tc.tile_pool(**kw))
        self.singles = p(name="singles", bufs=1)
        self.act = p(name="act", bufs=2)        # activation planes (z/cm/y/...)
        self.feat = p(name="feat", bufs=2)      # big bf16 feature buffers
        self.sfeat = p(name="sfeat", bufs=2)    # small (64p) feature buffers
        self.scr = p(name="scr", bufs=2)        # fp32 scratch (v planes, squares)
        self.tiny = p(name="tiny", bufs=8)      # stats vectors
        self.bfa = p(name="bfa", bufs=2)        # bf16 activation planes
        self.psum4 = p(name="psum4", bufs=5, space="PSUM")
        self.psum = p(name="psum", bufs=2, space="PSUM")
        self.psum1 = p(name="psum1", bufs=1, space="PSUM")

        self.ident = self.singles.tile([128, 128], F32)
        make_identity(nc, self.ident[:])
        self.ones = self.singles.tile([128, 128], F32)
        nc.gpsimd.memset(self.ones[:], 1.0)

    # ---- b-spline + silu feature construction --------------------------- #
    def kan_features(self, z, P, W, tag, split=1):
        """z: fp32 AP (P, W) flat view.  Returns (feat, sil):
        feat (P, 8, W) bf16 basis planes (x6 scale folded in weights),
        sil  (P, W) bf16 silu(z).  split>1 chops the free axis so work can
        start before the whole input plane is ready."""
        nc = self.nc
        pool = self.feat if P == 128 else self.sfeat
        spool = self.bfa if P == 128 else self.sfeat
        feat = pool.tile([P, 8, W], BF16, tag=f"feat_{128 if P == 128 else 64}")
        sg = self.scr.tile([P, W], F32, tag=f"sg_{128 if P == 128 else 64}")
        sil = spool.tile([P, W], BF16, tag=f"sil_{128 if P == 128 else 64}")
        S = W // split
        for s in range(split):
            # split>1 requires z shaped (P, split, S); whole-plane otherwise
            zs = z[:, s, :] if split > 1 else z
            nc.scalar.activation(sg[:, ts(s, S)], zs, AF.Sigmoid)
            nc.gpsimd.tensor_mul(sil[:, ts(s, S)], zs, sg[:, ts(s, S)])
            for g in range(8):
                v = self.scr.tile(
                    [P, S], F32, tag=f"v_{128 if P == 128 else 64}",
                    name=f"v{s}_{g}",
                )
                nc.vector._custom_dve(
                    VCLAMP, out=v[:], in0=zs, s0=2.5, s1=float(g) - 5.5,
                    imm2=float(g) - 1.5,
                )
                nc.vector._custom_dve(
                    BUMP3, out=feat[:, g, ts(s, S)], in0=v[:], s0=-4.0
                )
        return feat, sil

    # ---- matmul over features ------------------------------------------- #
    def kan_matmul_512(self, feat, sil, w, out_cb):
        """feat (128,8,1024), sil (128,1024), w (128,36,4,128) bf16 lhsT.
        For each m-tile: psum (128,256) after 36 accumulating matmuls ->
        out_cb(m, psum_ap)."""
        nc = self.nc
        pms = [
            self.psum4.tile([128, NTOK], F32, tag="pmm", name=f"pmm{m}")
            for m in range(4)
        ]
        gorder = [8] + list(range(8))
        for gi, g in enumerate(gorder):
            for k in range(4):
                rhs = sil[:, ts(k, NTOK)] if g == 8 else feat[:, g, ts(k, NTOK)]
                for m in range(4):
                    nc.tensor.matmul(
                        pms[m][:], w[:, g * 4 + k, m, :], rhs,
                        start=(gi == 0 and k == 0), stop=(gi == 8 and k == 3),
                    )
        for m in range(4):
            out_cb(m, pms[m])

    def kan_matmul_512_to_64(self, feat, sil, w):
        """-> psum (64, 256) after 36 matmuls. w (128, 36, 64)."""
        nc = self.nc
        pm = self.psum1.tile([64, NTOK], F32, tag="pk64")
        n = 0
        for g in [8] + list(range(8)):
            for k in range(4):
                rhs = sil[:, ts(k, NTOK)] if g == 8 else feat[:, g, ts(k, NTOK)]
                nc.tensor.matmul(
                    pm[:], w[:, g * 4 + k, :], rhs, start=(n == 0), stop=(n == 35)
                )
                n += 1
        return pm

    def kan_matmul_64_to_512(self, feat, sil, w, out_cb):
        """feat (64,8,256), sil (64,256), w (64,9,4,128)."""
        nc = self.nc
        pms = [
            self.psum4.tile([128, NTOK], F32, tag="pmm", name=f"pmm{m}")
            for m in range(4)
        ]
        gorder = [8] + list(range(8))
        for gi, g in enumerate(gorder):
            rhs = sil[:] if g == 8 else feat[:, g, :]
            for m in range(4):
                nc.tensor.matmul(
                    pms[m][:], w[:, g, m, :], rhs, start=(gi == 0), stop=(gi == 8)
                )
        for m in range(4):
            out_cb(m, pms[m])

    # ---- layernorm ------------------------------------------------------ #
    def stats_from(self, srcs):
        """srcs: list of (b, ap) free-dim slabs covering each batch; emits
        Identity+Square accum passes and returns stats tile (128, n) with
        layout [sum, sumsq] per accum slot plus the slot->batch map."""
        nc = self.nc
        n = len(srcs)
        stats = self.tiny.tile([128, 2 * n], F32, name="stats")
        for j, (b, sl) in enumerate(srcs):
            scr1 = self.scr.tile(list(sl.shape), F32, tag="sqscr", name=f"scr1_{j}")
            nc.scalar.activation(
                scr1[:], sl, AF.Identity, accum_out=stats[:, 2 * j : 2 * j + 1]
            )
            sqr = self.scr.tile(list(sl.shape), F32, tag="sqscr", name=f"sqr_{j}")
            nc.scalar.activation(
                sqr[:], sl, AF.Square, accum_out=stats[:, 2 * j + 1 : 2 * j + 2]
            )
        return stats

    def layernorm(self, xT, zname, lnw=None, lnb=None, stats=None, smap=None,
                  neng=None):
        """xT (128, 4, NTOK) fp32 -> z normalized per batch.  stats: tile
        (128, 2n) of [sum, sumsq] accum slots; smap[j] = batch of slot j
        (slots of one batch are summed)."""
        nc = self.nc
        if stats is None:
            stats = self.stats_from(
                [(b, xT[:, :, ts(b, C4)]) for b in range(BPC)]
            )
            smap = list(range(BPC))
        neng = neng or self.nc.gpsimd
        n2 = stats.shape[1]
        pstat = self.psum.tile([128, 128], F32, tag="ptr", name="pstat")[:, :n2]
        nc.tensor.matmul(pstat[:], self.ones[:], stats[:], start=True, stop=True)
        statsG = self.tiny.tile([128, n2], F32, name="statsG")
        nc.vector.tensor_scalar(
            out=statsG[:], in0=pstat[:], scalar1=INV_CNT, scalar2=None,
            op0=mybir.AluOpType.mult,
        )
        if len(smap) > BPC:
            # fold multiple slots per batch (pairwise into statsF)
            statsF = self.tiny.tile([128, 2 * BPC], F32, name="statsF")
            for b in range(BPC):
                idx = [j for j, bb in enumerate(smap) if bb == b]
                dst = statsF[:, 2 * b : 2 * b + 2]
                neng.tensor_add(
                    dst, statsG[:, 2 * idx[0] : 2 * idx[0] + 2],
                    statsG[:, 2 * idx[1] : 2 * idx[1] + 2],
                )
                for j in idx[2:]:
                    neng.tensor_add(dst, dst, statsG[:, 2 * j : 2 * j + 2])
        else:
            statsF = statsG
        mu = statsF[:, 0 : 2 * BPC : 2]
        e2 = statsF[:, 1 : 2 * BPC : 2]
        var = self.tiny.tile([128, BPC], F32)
        neng.tensor_mul(var[:], mu, mu)
        neng.tensor_sub(var[:], e2, var[:])
        a = self.tiny.tile([128, BPC], F32)
        neng.tensor_scalar_add(a[:], var[:], EPS)
        # y = rsqrt(a) by Newton from y0 = min(1/a, 1) (monotone from below)
        y = self.tiny.tile([128, BPC], F32)
        nc.vector.reciprocal(y[:], a[:])
        neng.tensor_scalar_min(y[:], y[:], 1.0)
        t = self.tiny.tile([128, BPC], F32)
        for _ in range(9):
            neng.tensor_mul(t[:], y[:], y[:])
            neng.tensor_mul(t[:], t[:], a[:])
            neng.tensor_scalar(
                out=t[:], in0=t[:], scalar1=-0.5, scalar2=1.5,
                op0=mybir.AluOpType.mult, op1=mybir.AluOpType.add,
            )
            neng.tensor_mul(y[:], y[:], t[:])
        musc = self.tiny.tile([128, BPC], F32)
        neng.tensor_mul(musc[:], mu, y[:])
        z = self.act.tile([128, 4, NTOK], F32, tag=zname)
        for b in range(BPC):
            nc.vector.tensor_scalar(
                out=z[:, :, ts(b, C4)], in0=xT[:, :, ts(b, C4)],
                scalar1=y[:, b : b + 1], scalar2=musc[:, b : b + 1],
                op0=mybir.AluOpType.mult, op1=mybir.AluOpType.subtract,
            )
        if lnw is not None:
            nc.vector.tensor_mul(z[:], z[:], lnw[:])
        if lnb is not None:
            nc.vector.tensor_add(z[:], z[:], lnb[:])
        return z

    # ---- gcn (folded) ---------------------------------------------------- #
    def gcn(self, tm_bf, wg, bias, bias_sc, yname, fin_gp=False):
        """tm_bf (128,4,NTOK) bf16; wg (128,4,4,128) bf16; bias (128,4) f32.
        Returns y (128,4,NTOK) f32 = gelu(tm @ Wg + b)."""
        nc = self.nc
        y = self.act.tile([128, 4, NTOK], F32, tag=yname)
        for m in range(4):
            pm = self.psum4.tile([128, NTOK], F32, tag="pmm")
            for k in range(4):
                nc.tensor.matmul(
                    pm[:], wg[:, k, m, :], tm_bf[:, k, :],
                    start=(k == 0), stop=(k == 3),
                )
            hb = self.scr.tile([128, NTOK], F32, tag="hb")
            nc.scalar.activation(hb[:], pm[:], AF.Identity, bias=bias[:, m : m + 1])
            e = self.scr.tile([128, NTOK], F32, tag="erf")
            nc.scalar.activation(
                e[:], pm[:], AF.Erf, bias=bias_sc[:, m : m + 1], scale=ISQ2
            )
            if fin_gp:
                t1 = self.scr.tile([128, NTOK], F32, tag="hb", name=f"gf{m}")
                nc.gpsimd.tensor_scalar_add(t1[:], e[:], 1.0)
                nc.gpsimd.tensor_mul(t1[:], t1[:], hb[:])
                nc.gpsimd.tensor_scalar(
                    out=y[:, m, :], in0=t1[:], scalar1=0.5, scalar2=None,
                    op0=mybir.AluOpType.mult,
                )
            else:
                nc.vector._custom_dve(
                    GELU_FIN, out=y[:, m, :], in0=hb[:], in1=e[:], s0=0.5
                )
        return y


def _emit(nc, ln_flags):
    """Emit the full per-core kernel.  ln_flags = (use_lnw1, use_lnb1,
    use_lnw2, use_lnb2) -- whether the TM layernorm affine params are
    non-trivial and must be applied."""
    use_lnw1, use_lnb1, use_lnw2, use_lnb2 = ln_flags
    dram = {}

    def din(name, shape, dt=BF16):
        dram[name] = nc.dram_tensor(name, shape, dt, kind="ExternalInput").ap()
        return dram[name]

    x_d = din("x_sh", (BPC, C4, T), F32)
    w_tm1k1 = din("w_tm1k1", (128, 36, 64))
    w_tm1k2 = din("w_tm1k2", (64, 9, 4, 128))
    w_k1 = din("w_k1", (128, 36, 4, 128))
    w_g1 = din("w_g1", (128, 4, 4, 128))
    b_g1 = din("b_g1", (128, 4, 2), F32)        # [:, :, 0]=b, [:, :, 1]=b/sqrt2
    w_tm2k1 = din("w_tm2k1", (128, 36, 64))
    w_tm2k2 = din("w_tm2k2", (64, 9, 4, 128))
    w_g2 = din("w_g2", (128, 4, 4, 128))
    b_g2 = din("b_g2", (128, 4, 2), F32)
    w_k2 = din("w_k2", (128, 36, 4, 128))
    ln1w_d = din("ln1w", (128, 4, NTOK), F32) if use_lnw1 else None
    ln1b_d = din("ln1b", (128, 4, NTOK), F32) if use_lnb1 else None
    ln2w_d = din("ln2w", (128, 4, NTOK), F32) if use_lnw2 else None
    ln2b_d = din("ln2b", (128, 4, NTOK), F32) if use_lnb2 else None
    out_d = nc.dram_tensor("out_sh", (BPC, C4, T), F32, kind="ExternalOutput").ap()

    with tile.TileContext(nc) as tc, ExitStack() as ctx:
        kb = _KB(nc, tc, ctx)
        wpool = ctx.enter_context(tc.tile_pool(name="weights", bufs=1))
        sync = nc.sync

        # ---- weight/input DMA (issue in consumption order) ----
        def wload(ap, shape, tag, dt=BF16):
            t = wpool.tile(list(shape), dt, tag=tag)
            sync.dma_start(t[:], ap)
            return t

        xN = kb.act.tile([C4, BPC, T], F32, tag="nat")
        x_r = x_d.rearrange("b p t -> p b t")
        for k in range(4):
            for b in range(BPC):
                sync.dma_start(
                    xN[:, b, ts(k, 128)], x_r[:, b, ts(k, 128)]
                )
        W_k1 = wload(w_k1, (128, 36, 4, 128), "wk_big")
        W_tm2k1 = wload(w_tm2k1, (128, 36, 64), "wtm2k1")
        W_tm2k2 = wload(w_tm2k2, (64, 9, 4, 128), "wtm2k2")
        W_g2 = wload(w_g2, (128, 4, 4, 128), "wg2")
        B_g2 = wload(b_g2, (128, 4, 2), "bg2", F32)
        W_tm1k1 = wload(w_tm1k1, (128, 36, 64), "wtm1k1")
        W_tm1k2 = wload(w_tm1k2, (64, 9, 4, 128), "wtm1k2")
        W_g1 = wload(w_g1, (128, 4, 4, 128), "wg1")
        B_g1 = wload(b_g1, (128, 4, 2), "bg1", F32)
        W_k2 = wload(w_k2, (128, 36, 4, 128), "wk_big")
        LN1W = wload(ln1w_d, (128, 4, NTOK), "ln1w", F32) if use_lnw1 else None
        LN1B = wload(ln1b_d, (128, 4, NTOK), "ln1b", F32) if use_lnb1 else None
        LN2W = wload(ln2w_d, (128, 4, NTOK), "ln2w", F32) if use_lnw2 else None
        LN2B = wload(ln2b_d, (128, 4, NTOK), "ln2b", F32) if use_lnb2 else None

        # ---- transpose x into T-layout ----
        xT = kb.act.tile([128, 4, NTOK], F32, tag="xT")
        for k in range(4):
            for b in range(BPC):
                pt = kb.psum.tile([128, 128], F32, tag="ptr")
                nc.tensor.transpose(pt[:], xN[:, b, ts(k, 128)], kb.ident[:])
                nc.scalar.copy(xT[:, k, ts(b, 128)], pt[:])

        # ---- TM1 stats from xN (ready before transposes finish) ----
        stats1 = kb.stats_from([(b, xN[:, b, :]) for b in range(BPC)])
        z1 = kb.layernorm(xT, "z", LN1W, LN1B, stats=stats1,
                          smap=list(range(BPC)))
        featX, silX = kb.kan_features(xT[:, :, :], 128, 4 * NTOK, "x", split=4)

        # ---- k1 matmuls -> cm (critical chain head) ----
        cm = kb.act.tile([128, 4, NTOK], F32, tag="cm")
        kb.kan_matmul_512(
            featX, silX, W_k1,
            lambda m, pm: nc.scalar.copy(cm[:, m, :], pm[:]),
        )

        # ---- TM1 kan chain + gcn1 (PE work emitted before the blocked
        #      tm2 matmuls so the in-order PE stream isn't inverted) ----
        featA, silA = kb.kan_features(z1[:, :, :], 128, 4 * NTOK, "a")
        p1 = kb.kan_matmul_512_to_64(featA, silA, W_tm1k1)
        z2 = kb.act.tile([64, NTOK], F32, tag="z64")
        nc.scalar.copy(z2[:], p1[:])
        featB, silB = kb.kan_features(z2[:, :], 64, NTOK, "b")
        tm1 = kb.bfa.tile([128, 4, NTOK], BF16, tag="tm")
        kb.kan_matmul_64_to_512(
            featB, silB, W_tm1k2,
            lambda m, pm: nc.vector.tensor_add(tm1[:, m, :], xT[:, m, :], pm[:]),
        )
        y1 = kb.gcn(tm1, W_g1, B_g1[:, :, 0], B_g1[:, :, 1], "y", fin_gp=True)

        # ---- TM2 on cm ----
        z3 = kb.layernorm(cm, "z", LN2W, LN2B)
        featC, silC = kb.kan_features(z3[:, :, :], 128, 4 * NTOK, "c")
        p2 = kb.kan_matmul_512_to_64(featC, silC, W_tm2k1)
        z4 = kb.act.tile([64, NTOK], F32, tag="z64")
        nc.scalar.copy(z4[:], p2[:])
        featD, silD = kb.kan_features(z4[:, :], 64, NTOK, "d")
        tm2 = kb.bfa.tile([128, 4, NTOK], BF16, tag="tm")
        kb.kan_matmul_64_to_512(
            featD, silD, W_tm2k2,
            lambda m, pm: nc.vector.tensor_add(tm2[:, m, :], cm[:, m, :], pm[:]),
        )
        y2 = kb.gcn(tm2, W_g2, B_g2[:, :, 0], B_g2[:, :, 1], "y")

        # ---- k2 on y2, final add, transpose out ----
        featY, silY = kb.kan_features(y2[:, :, :], 128, 4 * NTOK, "y")
        outT = kb.act.tile([128, 4, NTOK], F32, tag="outT")
        kb.kan_matmul_512(
            featY, silY, W_k2,
            lambda m, pm: nc.vector.tensor_add(outT[:, m, :], y1[:, m, :], pm[:]),
        )

        outN = kb.act.tile([C4, BPC, T], F32, tag="nat")
        out_r = out_d.rearrange("b p t -> p b t")
        for m in range(4):
            for b in range(BPC):
                pt = kb.psum.tile([128, 128], F32, tag="ptr")
                nc.tensor.transpose(pt[:], outT[:, m, ts(b, 128)], kb.ident[:])
                nc.scalar.copy(outN[:, b, ts(m, 128)], pt[:])
            sync.dma_start(
                out_r[:, :, ts(m, 128)], outN[:, :, ts(m, 128)]
            )

    return dram


def _build(ln_flags):
    key = ln_flags
    if key in _COMPILED:
        return _COMPILED[key]
    nc = bacc.Bacc("TRN2", target_bir_lowering=False, debug=False)
    _emit(nc, ln_flags)
    nc.compile()
    _COMPILED[key] = nc
    return nc


# --------------------------------------------------------------------------
# host-side weight preparation
# --------------------------------------------------------------------------
def _prep_kan_512(base_w, spline_w):
    """base_w (O,512), spline_w (O,512,8) -> (128, 36, O) or (128,36,4,128)."""
    O = base_w.shape[0]
    w = np.empty((128, 36, O), np.float32)
    for g in range(8):
        for k in range(4):
            # rows p -> channel 128k+p, feature basis g (x 1/6)
            w[:, g * 4 + k, :] = spline_w[:, k * 128 : (k + 1) * 128, g].T / 6.0
    for k in range(4):
        w[:, 32 + k, :] = base_w[:, k * 128 : (k + 1) * 128].T
    w = w.astype(BF)
    if O == 512:
        return np.ascontiguousarray(w.reshape(128, 36, 4, 128))
    return np.ascontiguousarray(w)


def _prep_kan_64(base_w, spline_w):
    """base_w (512,64), spline_w (512,64,8) -> (64, 9, 4, 128)."""
    w = np.empty((64, 9, 4, 128), np.float32)
    for g in range(8):
        for m in range(4):
            w[:, g, m, :] = spline_w[m * 128 : (m + 1) * 128, :, g].T / 6.0
    for m in range(4):
        w[:, 8, m, :] = base_w[m * 128 : (m + 1) * 128, :].T
    return np.ascontiguousarray(w.astype(BF))


def _prep_gcn(gw, gb):
    """gw (512, 1536) -> folded (128,4,4,128) bf16 lhsT; gb -> (128,4,2) f32."""
    Wf = gw[:, :512] + gw[:, 512:1024] + gw[:, 1024:]
    w = np.empty((128, 4, 4, 128), np.float32)
    for k in range(4):
        for m in range(4):
            w[:, k, m, :] = Wf[m * 128 : (m + 1) * 128, k * 128 : (k + 1) * 128].T
    b = np.empty((128, 4, 2), np.float32)
    b[:, :, 0] = gb.reshape(4, 128).T
    b[:, :, 1] = b[:, :, 0] * ISQ2
    return np.ascontiguousarray(w.astype(BF)), np.ascontiguousarray(b)


def _ln_plane(a):
    """ln param (512, 128) -> (128, 4, NTOK) f32 duplicated over batches."""
    p = np.empty((128, 4, NTOK), np.float32)
    for k in range(4):
        for b in range(BPC):
            p[:, k, b * C4 : (b + 1) * C4] = a[k * 128 : (k + 1) * 128, :]
    return np.ascontiguousarray(p)


def kernel(**inputs):
    i = {k: np.asarray(v) for k, v in inputs.items()}
    use_lnw1 = not np.all(i["tm1_ln_w"] == 1.0)
    use_lnb1 = not np.all(i["tm1_ln_b"] == 0.0)
    use_lnw2 = not np.all(i["tm_ln_w"] == 1.0)
    use_lnb2 = not np.all(i["tm_ln_b"] == 0.0)
    ln_flags = (use_lnw1, use_lnb1, use_lnw2, use_lnb2)
    nc = _build(ln_flags)

    w_tm1k1 = _prep_kan_512(i["tm1_k1_base"], i["tm1_k1_spline"])
    w_tm1k2 = _prep_kan_64(i["tm1_k2_base"], i["tm1_k2_spline"])
    w_k1 = _prep_kan_512(i["k1_base"], i["k1_spline"])
    w_g1, b_g1 = _prep_gcn(i["g1_w"], i["g1_b"])
    w_tm2k1 = _prep_kan_512(i["tm_k1_base"], i["tm_k1_spline"])
    w_tm2k2 = _prep_kan_64(i["tm_k2_base"], i["tm_k2_spline"])
    w_g2, b_g2 = _prep_gcn(i["g2_w"], i["g2_b"])
    w_k2 = _prep_kan_512(i["k2_base"], i["k2_spline"])

    shared = dict(
        w_tm1k1=w_tm1k1, w_tm1k2=w_tm1k2, w_k1=w_k1, w_g1=w_g1, b_g1=b_g1,
        w_tm2k1=w_tm2k1, w_tm2k2=w_tm2k2, w_g2=w_g2, b_g2=b_g2, w_k2=w_k2,
    )
    if use_lnw1:
        shared["ln1w"] = _ln_plane(i["tm1_ln_w"])
    if use_lnb1:
        shared["ln1b"] = _ln_plane(i["tm1_ln_b"])
    if use_lnw2:
        shared["ln2w"] = _ln_plane(i["tm_ln_w"])
    if use_lnb2:
        shared["ln2b"] = _ln_plane(i["tm_ln_b"])
    x = np.ascontiguousarray(i["x"], np.float32)
    in_maps = [
        {"x_sh": x[c * BPC : (c + 1) * BPC], **shared} for c in range(NCORES)
    ]
    res = run_bass_kernel_spmd(nc, in_maps, core_ids=list(range(NCORES)))
    out = np.empty((B, C4, T), np.float32)
    for c in range(NCORES):
        out[c * BPC : (c + 1) * BPC] = res.results[c]["out_sh"]
    return out


# revision 47
# speedup vs baseline: 1.2484x; 1.0118x over previous
"""Trainium2 Bass kernel for nn_Mixer2dTriUKAN_66417374265858.

Mathematical simplification: in gcn_spatial the adjacency enters only as
s = sum(softmax(P), axis=-1) == 1, so the entire FFT/prob_distance/softmax
branch cancels and gcn_spatial(x, a, w, b) == gelu(x @ (w1+w2+w3).T + b)
where w = [w1|w2|w3] split along the 3T axis.  (Verified: rel err ~9e-7.)

What remains per batch (B=16, C4=128 tokens, T=D=512):
  tm1 = TM(x)   = x + kan64->512(kan512->64(LN(x)))
  y1  = gelu(tm1 @ W1f.T + b1)
  cm  = kan512->512(x)
  tm2 = TM(cm)
  y2  = gelu(tm2 @ W2f.T + b2)
  out = y1 + kan512->512(y2)

kan(x) = silu(x) @ Wb.T + bspl(x) (.) Ws, with the 8 cubic B-spline bases
computed per element as basis_i(x) = (v^3 - 4*relu(v-1)^3)/6 where
v = relu(min(u-i, (i+4)-u)), u = 2.5x + 5.5 -- two fused custom DVE ops
(KAN_VCLAMP 5 ALU stages, KAN_BUMP3 8 stages) per basis.

Sharding: data-parallel over batch, 2 batches per core on 8 cores, weights
replicated.  All activations live in "transposed" layout (feature dim on
partitions, 256 = 2x128 tokens on the free axis); matmuls contract over the
partition axis with bf16 inputs and fp32 PSUM accumulation.
"""
from contextlib import ExitStack

import numpy as np
import ml_dtypes

import concourse.bacc as bacc
import concourse.bass as bass
import concourse.mybir as mybir
import concourse.tile as tile
from concourse.bass import ts
from concourse.bass_utils import run_bass_kernel_spmd
from concourse.masks import make_identity

import concourse.dve_ops as dve_ops
from concourse.dve_ops import DveOp
from concourse.dve_spec import Spec, Src0, Src1, C0, C1, C2, One, relu, sq, minn, lower
from concourse.dve_uop import DveOpSpec

BF = ml_dtypes.bfloat16
F32 = mybir.dt.float32
BF16 = mybir.dt.bfloat16
AF = mybir.ActivationFunctionType

B, C4, T = 16, 128, 512
NCORES = 8
BPC = B // NCORES          # batches per core
NTOK = BPC * C4            # 256 tokens on the free axis
INV_CNT = 1.0 / (C4 * T)   # layernorm element count per batch
EPS = 1e-5
ISQ2 = float(1.0 / np.sqrt(2.0))

_COMPILED = {}             # cache: key -> (nc, input_names)


# --------------------------------------------------------------------------
# custom DVE ops (registered at import, idempotent)
# --------------------------------------------------------------------------
def _mk_op(name, spec, subdim=False):
    shas = {}
    for ver in ("v3", "v4"):
        try:
            s = DveOpSpec(name=name, opcode=0, uops=lower(spec, ver=ver))
            shas[ver] = s.sha(ver)
        except Exception:
            pass
    return DveOp(name, spec, subdim=subdim, uops_sha=shas)


def _register_ops():
    have = {op.name for op in dve_ops.OPS}
    out = {}
    m = Src0 * C0
    _r = relu(Src0 - One)   # shared subexpression: computed once
    specs = {
        # relu(min(x*s0 - s1, imm2 - x*s0))
        "KAN_VCLAMP": Spec(
            body=relu(minn(m - C1, C2 - m)),
            reference=lambda in0, in1, s0, s1, imm2: np.maximum(
                np.minimum(in0 * s0 - s1, imm2 - in0 * s0), 0.0
            ),
        ),
        # v^3 + s0*relu(v-1)^3   (s0=-4)
        "KAN_BUMP3": Spec(
            body=sq(Src0) * Src0 + (sq(_r) * _r) * C0,
            reference=lambda in0, in1, s0, s1, imm2: in0**3
            + s0 * np.maximum(in0 - 1.0, 0.0) ** 3,
        ),
        # (in0*s0) * (in1 + 1)    -- gelu finish: 0.5*h*(1+erf(h/sqrt2))
        "GELU_FIN": Spec(
            body=(Src0 * C0) * (Src1 + One),
            reference=lambda in0, in1, s0, s1, imm2: (in0 * s0) * (in1 + 1.0),
        ),
        # ((in0+s1)*s0) * (in1 + 1)  -- gelu finish with fused bias, psum in0
        "GELU_FIN2": Spec(
            body=((Src0 + C1) * C0) * (Src1 + One),
            reference=lambda in0, in1, s0, s1, imm2: ((in0 + s1) * s0)
            * (in1 + 1.0),
        ),
    }
    for name, spec in specs.items():
        if name in have:
            out[name] = next(op for op in dve_ops.OPS if op.name == name)
            continue
        op = _mk_op(name, spec)
        dve_ops.OPS.append(op)
        dve_ops._SUB_OPCODE_FOR_NAME[name] = (
            dve_ops._CUSTOM_DVE_ROW_BASE + len(dve_ops.OPS) - 1
        )
        dve_ops.CUSTOM_DVE_SPECS[name] = spec
        out[name] = op
    return out


_OPS = _register_ops()
VCLAMP = _OPS["KAN_VCLAMP"]
GELU_FIN2 = _OPS["GELU_FIN2"]
BUMP3 = _OPS["KAN_BUMP3"]
GELU_FIN = _OPS["GELU_FIN"]


# --------------------------------------------------------------------------
# kernel builder
# --------------------------------------------------------------------------
class _KB:
    """Emission helper holding nc/tc/pools."""

    def __init__(self, nc, tc, ctx):
        self.nc = nc
        self.tc = tc
        p = lambda **kw: ctx.enter_context(tc.tile_pool(**kw))
        self.singles = p(name="singles", bufs=1)
        self.act = p(name="act", bufs=2)        # activation planes (z/cm/y/...)
        self.feat = p(name="feat", bufs=2)      # big bf16 feature buffers
        self.sfeat = p(name="sfeat", bufs=2)    # small (64p) feature buffers
        self.scr = p(name="scr", bufs=2)        # fp32 scratch (v planes, squares)
        self.tiny = p(name="tiny", bufs=8)      # stats vectors
        self.bfa = p(name="bfa", bufs=2)        # bf16 activation planes
        self.psum4 = p(name="psum4", bufs=5, space="PSUM")
        self.psum = p(name="psum", bufs=2, space="PSUM")
        self.psum1 = p(name="psum1", bufs=1, space="PSUM")

        self.ident = self.singles.tile([128, 128], F32)
        make_identity(nc, self.ident[:])
        self.ones = self.singles.tile([128, 128], F32)
        nc.gpsimd.memset(self.ones[:], 1.0)

    # ---- b-spline + silu feature construction --------------------------- #
    def kan_features(self, z, P, W, tag, split=1):
        """z: fp32 AP (P, W) flat view.  Returns (feat, sil):
        feat (P, 8, W) bf16 basis planes (x6 scale folded in weights),
        sil  (P, W) bf16 silu(z).  split>1 chops the free axis so work can
        start before the whole input plane is ready."""
        nc = self.nc
        pool = self.feat if P == 128 else self.sfeat
        spool = self.bfa if P == 128 else self.sfeat
        feat = pool.tile([P, 8, W], BF16, tag=f"feat_{128 if P == 128 else 64}")
        sg = self.scr.tile([P, W], F32, tag=f"sg_{128 if P == 128 else 64}")
        sil = spool.tile([P, W], BF16, tag=f"sil_{128 if P == 128 else 64}")
        S = W // split
        for s in range(split):
            # split>1 requires z shaped (P, split, S); whole-plane otherwise
            zs = z[:, s, :] if split > 1 else z
            nc.scalar.activation(sg[:, ts(s, S)], zs, AF.Sigmoid)
            nc.gpsimd.tensor_mul(sil[:, ts(s, S)], zs, sg[:, ts(s, S)])
            for g in range(8):
                v = self.scr.tile(
                    [P, S], F32, tag=f"v_{128 if P == 128 else 64}",
                    name=f"v{s}_{g}",
                )
                nc.vector._custom_dve(
                    VCLAMP, out=v[:], in0=zs, s0=2.5, s1=float(g) - 5.5,
                    imm2=float(g) - 1.5,
                )
                nc.vector._custom_dve(
                    BUMP3, out=feat[:, g, ts(s, S)], in0=v[:], s0=-4.0
                )
        return feat, sil

    # ---- matmul over features ------------------------------------------- #
    def kan_matmul_512(self, feat, sil, w, out_cb):
        """feat (128,8,1024), sil (128,1024), w (128,36,4,128) bf16 lhsT.
        For each m-tile: psum (128,256) after 36 accumulating matmuls ->
        out_cb(m, psum_ap)."""
        nc = self.nc
        pms = [
            self.psum4.tile([128, NTOK], F32, tag="pmm", name=f"pmm{m}")
            for m in range(4)
        ]
        gorder = [8] + list(range(8))
        for gi, g in enumerate(gorder):
            for k in range(4):
                rhs = sil[:, ts(k, NTOK)] if g == 8 else feat[:, g, ts(k, NTOK)]
                for m in range(4):
                    nc.tensor.matmul(
                        pms[m][:], w[:, g * 4 + k, m, :], rhs,
                        start=(gi == 0 and k == 0), stop=(gi == 8 and k == 3),
                    )
        for m in range(4):
            out_cb(m, pms[m])

    def kan_matmul_512_to_64(self, feat, sil, w):
        """-> psum (64, 256) after 36 matmuls. w (128, 36, 64)."""
        nc = self.nc
        pm = self.psum1.tile([64, NTOK], F32, tag="pk64")
        n = 0
        for g in [8] + list(range(8)):
            for k in range(4):
                rhs = sil[:, ts(k, NTOK)] if g == 8 else feat[:, g, ts(k, NTOK)]
                nc.tensor.matmul(
                    pm[:], w[:, g * 4 + k, :], rhs, start=(n == 0), stop=(n == 35)
                )
                n += 1
        return pm

    def kan_matmul_64_to_512(self, feat, sil, w, out_cb):
        """feat (64,8,256), sil (64,256), w (64,9,4,128)."""
        nc = self.nc
        pms = [
            self.psum4.tile([128, NTOK], F32, tag="pmm", name=f"pmm{m}")
            for m in range(4)
        ]
        gorder = [8] + list(range(8))
        for gi, g in enumerate(gorder):
            rhs = sil[:] if g == 8 else feat[:, g, :]
            for m in range(4):
                nc.tensor.matmul(
                    pms[m][:], w[:, g, m, :], rhs, start=(gi == 0), stop=(gi == 8)
                )
        for m in range(4):
            out_cb(m, pms[m])

    # ---- layernorm ------------------------------------------------------ #
    def stats_from(self, srcs):
        """srcs: list of (b, ap) free-dim slabs covering each batch; emits
        Identity+Square accum passes and returns stats tile (128, n) with
        layout [sum, sumsq] per accum slot plus the slot->batch map."""
        nc = self.nc
        n = len(srcs)
        stats = self.tiny.tile([128, 2 * n], F32, name="stats")
        for j, (b, sl) in enumerate(srcs):
            scr1 = self.scr.tile(list(sl.shape), F32, tag="sqscr", name=f"scr1_{j}")
            nc.scalar.activation(
                scr1[:], sl, AF.Identity, accum_out=stats[:, 2 * j : 2 * j + 1]
            )
            sqr = self.scr.tile(list(sl.shape), F32, tag="sqscr", name=f"sqr_{j}")
            nc.scalar.activation(
                sqr[:], sl, AF.Square, accum_out=stats[:, 2 * j + 1 : 2 * j + 2]
            )
        return stats

    def layernorm(self, xT, zname, lnw=None, lnb=None, stats=None, smap=None,
                  neng=None):
        """xT (128, 4, NTOK) fp32 -> z normalized per batch.  stats: tile
        (128, 2n) of [sum, sumsq] accum slots; smap[j] = batch of slot j
        (slots of one batch are summed)."""
        nc = self.nc
        if stats is None:
            stats = self.stats_from(
                [(b, xT[:, :, ts(b, C4)]) for b in range(BPC)]
            )
            smap = list(range(BPC))
        neng = neng or self.nc.gpsimd
        n2 = stats.shape[1]
        pstat = self.psum.tile([128, 128], F32, tag="ptr", name="pstat")[:, :n2]
        nc.tensor.matmul(pstat[:], self.ones[:], stats[:], start=True, stop=True)
        statsG = self.tiny.tile([128, n2], F32, name="statsG")
        nc.vector.tensor_scalar(
            out=statsG[:], in0=pstat[:], scalar1=INV_CNT, scalar2=None,
            op0=mybir.AluOpType.mult,
        )
        if len(smap) > BPC:
            # fold multiple slots per batch (pairwise into statsF)
            statsF = self.tiny.tile([128, 2 * BPC], F32, name="statsF")
            for b in range(BPC):
                idx = [j for j, bb in enumerate(smap) if bb == b]
                dst = statsF[:, 2 * b : 2 * b + 2]
                neng.tensor_add(
                    dst, statsG[:, 2 * idx[0] : 2 * idx[0] + 2],
                    statsG[:, 2 * idx[1] : 2 * idx[1] + 2],
                )
                for j in idx[2:]:
                    neng.tensor_add(dst, dst, statsG[:, 2 * j : 2 * j + 2])
        else:
            statsF = statsG
        mu = statsF[:, 0 : 2 * BPC : 2]
        e2 = statsF[:, 1 : 2 * BPC : 2]
        var = self.tiny.tile([128, BPC], F32)
        neng.tensor_mul(var[:], mu, mu)
        neng.tensor_sub(var[:], e2, var[:])
        a = self.tiny.tile([128, BPC], F32)
        neng.tensor_scalar_add(a[:], var[:], EPS)
        # y = rsqrt(a) by Newton from y0 = min(1/a, 1) (monotone from below)
        y = self.tiny.tile([128, BPC], F32)
        nc.vector.reciprocal(y[:], a[:])
        neng.tensor_scalar_min(y[:], y[:], 1.0)
        t = self.tiny.tile([128, BPC], F32)
        for _ in range(9):
            neng.tensor_mul(t[:], y[:], y[:])
            neng.tensor_mul(t[:], t[:], a[:])
            neng.tensor_scalar(
                out=t[:], in0=t[:], scalar1=-0.5, scalar2=1.5,
                op0=mybir.AluOpType.mult, op1=mybir.AluOpType.add,
            )
            neng.tensor_mul(y[:], y[:], t[:])
        musc = self.tiny.tile([128, BPC], F32)
        neng.tensor_mul(musc[:], mu, y[:])
        z = self.act.tile([128, 4, NTOK], F32, tag=zname)
        for b in range(BPC):
            nc.vector.tensor_scalar(
                out=z[:, :, ts(b, C4)], in0=xT[:, :, ts(b, C4)],
                scalar1=y[:, b : b + 1], scalar2=musc[:, b : b + 1],
                op0=mybir.AluOpType.mult, op1=mybir.AluOpType.subtract,
            )
        if lnw is not None:
            nc.vector.tensor_mul(z[:], z[:], lnw[:])
        if lnb is not None:
            nc.vector.tensor_add(z[:], z[:], lnb[:])
        return z

    # ---- gcn (folded) ---------------------------------------------------- #
    def gcn(self, tm_bf, wg, bias, bias_sc, yname, fin_gp=False):
        """tm_bf (128,4,NTOK) bf16; wg (128,4,4,128) bf16; bias (128,4) f32.
        Returns y (128,4,NTOK) f32 = gelu(tm @ Wg + b)."""
        nc = self.nc
        y = self.act.tile([128, 4, NTOK], F32, tag=yname)
        for m in range(4):
            pm = self.psum4.tile([128, NTOK], F32, tag="pmm")
            for k in range(4):
                nc.tensor.matmul(
                    pm[:], wg[:, k, m, :], tm_bf[:, k, :],
                    start=(k == 0), stop=(k == 3),
                )
            e = self.scr.tile([128, NTOK], F32, tag="erf")
            nc.scalar.activation(
                e[:], pm[:], AF.Erf, bias=bias_sc[:, m : m + 1], scale=ISQ2
            )
            if fin_gp:
                hb = self.scr.tile([128, NTOK], F32, tag="hb", name=f"hb{m}")
                nc.scalar.activation(
                    hb[:], pm[:], AF.Identity, bias=bias[:, m : m + 1]
                )
                t1 = self.scr.tile([128, NTOK], F32, tag="hb", name=f"gf{m}")
                nc.gpsimd.tensor_scalar_add(t1[:], e[:], 1.0)
                nc.gpsimd.tensor_mul(t1[:], t1[:], hb[:])
                nc.gpsimd.tensor_scalar(
                    out=y[:, m, :], in0=t1[:], scalar1=0.5, scalar2=None,
                    op0=mybir.AluOpType.mult,
                )
            else:
                nc.vector._custom_dve(
                    GELU_FIN2, out=y[:, m, :], in0=pm[:], in1=e[:], s0=0.5,
                    s1=bias[:, m : m + 1],
                )
        return y


def _emit(nc, ln_flags):
    """Emit the full per-core kernel.  ln_flags = (use_lnw1, use_lnb1,
    use_lnw2, use_lnb2) -- whether the TM layernorm affine params are
    non-trivial and must be applied."""
    use_lnw1, use_lnb1, use_lnw2, use_lnb2 = ln_flags
    dram = {}

    def din(name, shape, dt=BF16):
        dram[name] = nc.dram_tensor(name, shape, dt, kind="ExternalInput").ap()
        return dram[name]

    x_d = din("x_sh", (BPC, C4, T), F32)
    w_tm1k1 = din("w_tm1k1", (128, 36, 64))
    w_tm1k2 = din("w_tm1k2", (64, 9, 4, 128))
    w_k1 = din("w_k1", (128, 36, 4, 128))
    w_g1 = din("w_g1", (128, 4, 4, 128))
    b_g1 = din("b_g1", (128, 4, 2), F32)        # [:, :, 0]=b, [:, :, 1]=b/sqrt2
    w_tm2k1 = din("w_tm2k1", (128, 36, 64))
    w_tm2k2 = din("w_tm2k2", (64, 9, 4, 128))
    w_g2 = din("w_g2", (128, 4, 4, 128))
    b_g2 = din("b_g2", (128, 4, 2), F32)
    w_k2 = din("w_k2", (128, 36, 4, 128))
    ln1w_d = din("ln1w", (128, 4, NTOK)) if use_lnw1 else None
    ln1b_d = din("ln1b", (128, 4, NTOK)) if use_lnb1 else None
    ln2w_d = din("ln2w", (128, 4, NTOK)) if use_lnw2 else None
    ln2b_d = din("ln2b", (128, 4, NTOK)) if use_lnb2 else None
    out_d = nc.dram_tensor("out_sh", (BPC, C4, T), F32, kind="ExternalOutput").ap()

    with tile.TileContext(nc) as tc, ExitStack() as ctx:
        kb = _KB(nc, tc, ctx)
        wpool = ctx.enter_context(tc.tile_pool(name="weights", bufs=1))
        sync = nc.sync

        # ---- weight/input DMA (issue in consumption order) ----
        def wload(ap, shape, tag, dt=BF16):
            t = wpool.tile(list(shape), dt, tag=tag)
            sync.dma_start(t[:], ap)
            return t

        xN = kb.act.tile([C4, BPC, T], F32, tag="nat")
        x_r = x_d.rearrange("b p t -> p b t")
        for k in range(4):
            for b in range(BPC):
                sync.dma_start(
                    xN[:, b, ts(k, 128)], x_r[:, b, ts(k, 128)]
                )
        W_k1 = wload(w_k1, (128, 36, 4, 128), "wk_big")
        W_tm2k1 = wload(w_tm2k1, (128, 36, 64), "wtm2k1")
        W_tm2k2 = wload(w_tm2k2, (64, 9, 4, 128), "wtm2k2")
        W_g2 = wload(w_g2, (128, 4, 4, 128), "wg2")
        B_g2 = wload(b_g2, (128, 4, 2), "bg2", F32)
        W_tm1k1 = wload(w_tm1k1, (128, 36, 64), "wtm1k1")
        W_tm1k2 = wload(w_tm1k2, (64, 9, 4, 128), "wtm1k2")
        W_g1 = wload(w_g1, (128, 4, 4, 128), "wg1")
        B_g1 = wload(b_g1, (128, 4, 2), "bg1", F32)
        W_k2 = wload(w_k2, (128, 36, 4, 128), "wk_big")
        LN1W = wload(ln1w_d, (128, 4, NTOK), "ln1w") if use_lnw1 else None
        LN1B = wload(ln1b_d, (128, 4, NTOK), "ln1b") if use_lnb1 else None
        LN2W = wload(ln2w_d, (128, 4, NTOK), "ln2w") if use_lnw2 else None
        LN2B = wload(ln2b_d, (128, 4, NTOK), "ln2b") if use_lnb2 else None

        # ---- transpose x into T-layout ----
        xT = kb.act.tile([128, 4, NTOK], F32, tag="xT")
        for k in range(4):
            for b in range(BPC):
                pt = kb.psum.tile([128, 128], F32, tag="ptr")
                nc.tensor.transpose(pt[:], xN[:, b, ts(k, 128)], kb.ident[:])
                nc.scalar.copy(xT[:, k, ts(b, 128)], pt[:])

        # ---- TM1 stats from xN (ready before transposes finish) ----
        stats1 = kb.stats_from([(b, xN[:, b, :]) for b in range(BPC)])
        z1 = kb.layernorm(xT, "z", LN1W, LN1B, stats=stats1,
                          smap=list(range(BPC)))
        featX, silX = kb.kan_features(xT[:, :, :], 128, 4 * NTOK, "x", split=4)

        # ---- k1 matmuls -> cm (critical chain head) ----
        cm = kb.act.tile([128, 4, NTOK], F32, tag="cm")
        kb.kan_matmul_512(
            featX, silX, W_k1,
            lambda m, pm: nc.scalar.copy(cm[:, m, :], pm[:]),
        )

        # ---- TM1 kan chain + gcn1 (PE work emitted before the blocked
        #      tm2 matmuls so the in-order PE stream isn't inverted) ----
        featA, silA = kb.kan_features(z1[:, :, :], 128, 4 * NTOK, "a")
        p1 = kb.kan_matmul_512_to_64(featA, silA, W_tm1k1)
        z2 = kb.act.tile([64, NTOK], F32, tag="z64")
        nc.scalar.copy(z2[:], p1[:])
        featB, silB = kb.kan_features(z2[:, :], 64, NTOK, "b")
        tm1 = kb.bfa.tile([128, 4, NTOK], BF16, tag="tm")
        kb.kan_matmul_64_to_512(
            featB, silB, W_tm1k2,
            lambda m, pm: nc.vector.tensor_add(tm1[:, m, :], xT[:, m, :], pm[:]),
        )
        y1 = kb.gcn(tm1, W_g1, B_g1[:, :, 0], B_g1[:, :, 1], "y", fin_gp=True)

        # ---- TM2 on cm ----
        z3 = kb.layernorm(cm, "z", LN2W, LN2B)
        featC, silC = kb.kan_features(z3[:, :, :], 128, 4 * NTOK, "c")
        p2 = kb.kan_matmul_512_to_64(featC, silC, W_tm2k1)
        z4 = kb.act.tile([64, NTOK], F32, tag="z64")
        nc.scalar.copy(z4[:], p2[:])
        featD, silD = kb.kan_features(z4[:, :], 64, NTOK, "d")
        tm2 = kb.bfa.tile([128, 4, NTOK], BF16, tag="tm")
        kb.kan_matmul_64_to_512(
            featD, silD, W_tm2k2,
            lambda m, pm: nc.vector.tensor_add(tm2[:, m, :], cm[:, m, :], pm[:]),
        )
        y2 = kb.gcn(tm2, W_g2, B_g2[:, :, 0], B_g2[:, :, 1], "y")

        # ---- k2 on y2, final add, transpose out ----
        featY, silY = kb.kan_features(y2[:, :, :], 128, 4 * NTOK, "y")
        outT = kb.act.tile([128, 4, NTOK], F32, tag="outT")
        kb.kan_matmul_512(
            featY, silY, W_k2,
            lambda m, pm: nc.vector.tensor_add(outT[:, m, :], y1[:, m, :], pm[:]),
        )

        outN = kb.act.tile([C4, BPC, T], F32, tag="nat")
        out_r = out_d.rearrange("b p t -> p b t")
        for m in range(4):
            for b in range(BPC):
                pt = kb.psum.tile([128, 128], F32, tag="ptr")
                nc.tensor.transpose(pt[:], outT[:, m, ts(b, 128)], kb.ident[:])
                nc.scalar.copy(outN[:, b, ts(m, 128)], pt[:])
            sync.dma_start(
                out_r[:, :, ts(m, 128)], outN[:, :, ts(m, 128)]
            )

    return dram


def _build(ln_flags):
    key = ln_flags
    if key in _COMPILED:
        return _COMPILED[key]
    nc = bacc.Bacc("TRN2", target_bir_lowering=False, debug=False)
    _emit(nc, ln_flags)
    nc.compile()
    _COMPILED[key] = nc
    return nc


# --------------------------------------------------------------------------
# host-side weight preparation
# --------------------------------------------------------------------------
def _prep_kan_512(base_w, spline_w):
    """base_w (O,512), spline_w (O,512,8) -> (128, 36, O) or (128,36,4,128)."""
    O = base_w.shape[0]
    w = np.empty((128, 36, O), np.float32)
    for g in range(8):
        for k in range(4):
            # rows p -> channel 128k+p, feature basis g (x 1/6)
            w[:, g * 4 + k, :] = spline_w[:, k * 128 : (k + 1) * 128, g].T / 6.0
    for k in range(4):
        w[:, 32 + k, :] = base_w[:, k * 128 : (k + 1) * 128].T
    w = w.astype(BF)
    if O == 512:
        return np.ascontiguousarray(w.reshape(128, 36, 4, 128))
    return np.ascontiguousarray(w)


def _prep_kan_64(base_w, spline_w):
    """base_w (512,64), spline_w (512,64,8) -> (64, 9, 4, 128)."""
    w = np.empty((64, 9, 4, 128), np.float32)
    for g in range(8):
        for m in range(4):
            w[:, g, m, :] = spline_w[m * 128 : (m + 1) * 128, :, g].T / 6.0
    for m in range(4):
        w[:, 8, m, :] = base_w[m * 128 : (m + 1) * 128, :].T
    return np.ascontiguousarray(w.astype(BF))


def _prep_gcn(gw, gb):
    """gw (512, 1536) -> folded (128,4,4,128) bf16 lhsT; gb -> (128,4,2) f32."""
    Wf = gw[:, :512] + gw[:, 512:1024] + gw[:, 1024:]
    w = np.empty((128, 4, 4, 128), np.float32)
    for k in range(4):
        for m in range(4):
            w[:, k, m, :] = Wf[m * 128 : (m + 1) * 128, k * 128 : (k + 1) * 128].T
    b = np.empty((128, 4, 2), np.float32)
    b[:, :, 0] = gb.reshape(4, 128).T
    b[:, :, 1] = b[:, :, 0] * ISQ2
    return np.ascontiguousarray(w.astype(BF)), np.ascontiguousarray(b)


def _ln_plane(a):
    """ln param (512, 128) -> (128, 4, NTOK) bf16 duplicated over batches."""
    p = np.empty((128, 4, NTOK), np.float32)
    for k in range(4):
        for b in range(BPC):
            p[:, k, b * C4 : (b + 1) * C4] = a[k * 128 : (k + 1) * 128, :]
    return np.ascontiguousarray(p.astype(BF))


def kernel(**inputs):
    i = {k: np.asarray(v) for k, v in inputs.items()}
    use_lnw1 = not np.all(i["tm1_ln_w"] == 1.0)
    use_lnb1 = not np.all(i["tm1_ln_b"] == 0.0)
    use_lnw2 = not np.all(i["tm_ln_w"] == 1.0)
    use_lnb2 = not np.all(i["tm_ln_b"] == 0.0)
    ln_flags = (use_lnw1, use_lnb1, use_lnw2, use_lnb2)
    nc = _build(ln_flags)

    w_tm1k1 = _prep_kan_512(i["tm1_k1_base"], i["tm1_k1_spline"])
    w_tm1k2 = _prep_kan_64(i["tm1_k2_base"], i["tm1_k2_spline"])
    w_k1 = _prep_kan_512(i["k1_base"], i["k1_spline"])
    w_g1, b_g1 = _prep_gcn(i["g1_w"], i["g1_b"])
    w_tm2k1 = _prep_kan_512(i["tm_k1_base"], i["tm_k1_spline"])
    w_tm2k2 = _prep_kan_64(i["tm_k2_base"], i["tm_k2_spline"])
    w_g2, b_g2 = _prep_gcn(i["g2_w"], i["g2_b"])
    w_k2 = _prep_kan_512(i["k2_base"], i["k2_spline"])

    shared = dict(
        w_tm1k1=w_tm1k1, w_tm1k2=w_tm1k2, w_k1=w_k1, w_g1=w_g1, b_g1=b_g1,
        w_tm2k1=w_tm2k1, w_tm2k2=w_tm2k2, w_g2=w_g2, b_g2=b_g2, w_k2=w_k2,
    )
    if use_lnw1:
        shared["ln1w"] = _ln_plane(i["tm1_ln_w"])
    if use_lnb1:
        shared["ln1b"] = _ln_plane(i["tm1_ln_b"])
    if use_lnw2:
        shared["ln2w"] = _ln_plane(i["tm_ln_w"])
    if use_lnb2:
        shared["ln2b"] = _ln_plane(i["tm_ln_b"])
    x = np.ascontiguousarray(i["x"], np.float32)
    in_maps = [
        {"x_sh": x[c * BPC : (c + 1) * BPC], **shared} for c in range(NCORES)
    ]
    res = run_bass_kernel_spmd(nc, in_maps, core_ids=list(range(NCORES)))
    out = np.empty((B, C4, T), np.float32)
    for c in range(NCORES):
        out[c * BPC : (c + 1) * BPC] = res.results[c]["out_sh"]
    return out


# revision 48
# speedup vs baseline: 1.2549x; 1.0052x over previous
"""Trainium2 Bass kernel for nn_Mixer2dTriUKAN_66417374265858.

Mathematical simplification: in gcn_spatial the adjacency enters only as
s = sum(softmax(P), axis=-1) == 1, so the entire FFT/prob_distance/softmax
branch cancels and gcn_spatial(x, a, w, b) == gelu(x @ (w1+w2+w3).T + b)
where w = [w1|w2|w3] split along the 3T axis.  (Verified: rel err ~9e-7.)

What remains per batch (B=16, C4=128 tokens, T=D=512):
  tm1 = TM(x)   = x + kan64->512(kan512->64(LN(x)))
  y1  = gelu(tm1 @ W1f.T + b1)
  cm  = kan512->512(x)
  tm2 = TM(cm)
  y2  = gelu(tm2 @ W2f.T + b2)
  out = y1 + kan512->512(y2)

kan(x) = silu(x) @ Wb.T + bspl(x) (.) Ws, with the 8 cubic B-spline bases
computed per element as basis_i(x) = (v^3 - 4*relu(v-1)^3)/6 where
v = relu(min(u-i, (i+4)-u)), u = 2.5x + 5.5 -- two fused custom DVE ops
(KAN_VCLAMP 5 ALU stages, KAN_BUMP3 8 stages) per basis.

Sharding: data-parallel over batch, 2 batches per core on 8 cores, weights
replicated.  All activations live in "transposed" layout (feature dim on
partitions, 256 = 2x128 tokens on the free axis); matmuls contract over the
partition axis with bf16 inputs and fp32 PSUM accumulation.
"""
from contextlib import ExitStack

import numpy as np
import ml_dtypes

import concourse.bacc as bacc
import concourse.bass as bass
import concourse.mybir as mybir
import concourse.tile as tile
from concourse.bass import ts
from concourse.bass_utils import run_bass_kernel_spmd
from concourse.masks import make_identity

import concourse.dve_ops as dve_ops
from concourse.dve_ops import DveOp
from concourse.dve_spec import Spec, Src0, Src1, C0, C1, C2, One, relu, sq, minn, lower
from concourse.dve_uop import DveOpSpec

BF = ml_dtypes.bfloat16
F32 = mybir.dt.float32
BF16 = mybir.dt.bfloat16
AF = mybir.ActivationFunctionType

B, C4, T = 16, 128, 512
NCORES = 8
BPC = B // NCORES          # batches per core
NTOK = BPC * C4            # 256 tokens on the free axis
INV_CNT = 1.0 / (C4 * T)   # layernorm element count per batch
EPS = 1e-5
ISQ2 = float(1.0 / np.sqrt(2.0))

_COMPILED = {}             # cache: key -> (nc, input_names)


# --------------------------------------------------------------------------
# custom DVE ops (registered at import, idempotent)
# --------------------------------------------------------------------------
def _mk_op(name, spec, subdim=False):
    shas = {}
    for ver in ("v3", "v4"):
        try:
            s = DveOpSpec(name=name, opcode=0, uops=lower(spec, ver=ver))
            shas[ver] = s.sha(ver)
        except Exception:
            pass
    return DveOp(name, spec, subdim=subdim, uops_sha=shas)


def _register_ops():
    have = {op.name for op in dve_ops.OPS}
    out = {}
    m = Src0 * C0
    _r = relu(Src0 - One)   # shared subexpression: computed once
    specs = {
        # relu(min(x*s0 - s1, imm2 - x*s0))
        "KAN_VCLAMP": Spec(
            body=relu(minn(m - C1, C2 - m)),
            reference=lambda in0, in1, s0, s1, imm2: np.maximum(
                np.minimum(in0 * s0 - s1, imm2 - in0 * s0), 0.0
            ),
        ),
        # v^3 + s0*relu(v-1)^3   (s0=-4)
        "KAN_BUMP3": Spec(
            body=sq(Src0) * Src0 + (sq(_r) * _r) * C0,
            reference=lambda in0, in1, s0, s1, imm2: in0**3
            + s0 * np.maximum(in0 - 1.0, 0.0) ** 3,
        ),
        # (in0*s0) * (in1 + 1)    -- gelu finish: 0.5*h*(1+erf(h/sqrt2))
        "GELU_FIN": Spec(
            body=(Src0 * C0) * (Src1 + One),
            reference=lambda in0, in1, s0, s1, imm2: (in0 * s0) * (in1 + 1.0),
        ),
        # ((in0+s1)*s0) * (in1 + 1)  -- gelu finish with fused bias, psum in0
        "GELU_FIN2": Spec(
            body=((Src0 + C1) * C0) * (Src1 + One),
            reference=lambda in0, in1, s0, s1, imm2: ((in0 + s1) * s0)
            * (in1 + 1.0),
        ),
    }
    for name, spec in specs.items():
        if name in have:
            out[name] = next(op for op in dve_ops.OPS if op.name == name)
            continue
        op = _mk_op(name, spec)
        dve_ops.OPS.append(op)
        dve_ops._SUB_OPCODE_FOR_NAME[name] = (
            dve_ops._CUSTOM_DVE_ROW_BASE + len(dve_ops.OPS) - 1
        )
        dve_ops.CUSTOM_DVE_SPECS[name] = spec
        out[name] = op
    return out


_OPS = _register_ops()
VCLAMP = _OPS["KAN_VCLAMP"]
GELU_FIN2 = _OPS["GELU_FIN2"]
BUMP3 = _OPS["KAN_BUMP3"]
GELU_FIN = _OPS["GELU_FIN"]


# --------------------------------------------------------------------------
# kernel builder
# --------------------------------------------------------------------------
class _KB:
    """Emission helper holding nc/tc/pools."""

    def __init__(self, nc, tc, ctx):
        self.nc = nc
        self.tc = tc
        p = lambda **kw: ctx.enter_context(tc.tile_pool(**kw))
        self.singles = p(name="singles", bufs=1)
        self.act = p(name="act", bufs=2)        # activation planes (z/cm/y/...)
        self.feat = p(name="feat", bufs=2)      # big bf16 feature buffers
        self.sfeat = p(name="sfeat", bufs=2)    # small (64p) feature buffers
        self.scr = p(name="scr", bufs=2)        # fp32 scratch (v planes, squares)
        self.tiny = p(name="tiny", bufs=8)      # stats vectors
        self.bfa = p(name="bfa", bufs=2)        # bf16 activation planes
        self.psum4 = p(name="psum4", bufs=5, space="PSUM")
        self.psum = p(name="psum", bufs=2, space="PSUM")
        self.psum1 = p(name="psum1", bufs=1, space="PSUM")

        self.ident = self.singles.tile([128, 128], F32)
        make_identity(nc, self.ident[:])
        self.ones = self.singles.tile([128, 128], F32)
        nc.gpsimd.memset(self.ones[:], 1.0)

    # ---- b-spline + silu feature construction --------------------------- #
    def kan_features(self, z, P, W, tag, split=1):
        """z: fp32 AP (P, W) flat view.  Returns (feat, sil):
        feat (P, 8, W) bf16 basis planes (x6 scale folded in weights),
        sil  (P, W) bf16 silu(z).  split>1 chops the free axis so work can
        start before the whole input plane is ready."""
        nc = self.nc
        pool = self.feat if P == 128 else self.sfeat
        spool = self.bfa if P == 128 else self.sfeat
        feat = pool.tile([P, 8, W], BF16, tag=f"feat_{128 if P == 128 else 64}")
        sg = self.scr.tile([P, W], F32, tag=f"sg_{128 if P == 128 else 64}")
        sil = spool.tile([P, W], BF16, tag=f"sil_{128 if P == 128 else 64}")
        S = W // split
        for s in range(split):
            # split>1 requires z shaped (P, split, S); whole-plane otherwise
            zs = z[:, s, :] if split > 1 else z
            nc.scalar.activation(sg[:, ts(s, S)], zs, AF.Sigmoid)
            nc.gpsimd.tensor_mul(sil[:, ts(s, S)], zs, sg[:, ts(s, S)])
            for g in range(8):
                v = self.scr.tile(
                    [P, S], F32, tag=f"v_{128 if P == 128 else 64}",
                    name=f"v{s}_{g}",
                )
                nc.vector._custom_dve(
                    VCLAMP, out=v[:], in0=zs, s0=2.5, s1=float(g) - 5.5,
                    imm2=float(g) - 1.5,
                )
                nc.vector._custom_dve(
                    BUMP3, out=feat[:, g, ts(s, S)], in0=v[:], s0=-4.0
                )
        return feat, sil

    # ---- matmul over features ------------------------------------------- #
    def kan_matmul_512(self, feat, sil, w, out_cb):
        """feat (128,8,1024), sil (128,1024), w (128,36,4,128) bf16 lhsT.
        For each m-tile: psum (128,256) after 36 accumulating matmuls ->
        out_cb(m, psum_ap)."""
        nc = self.nc
        pms = [
            self.psum4.tile([128, NTOK], F32, tag="pmm", name=f"pmm{m}")
            for m in range(4)
        ]
        gorder = [8] + list(range(8))
        for gi, g in enumerate(gorder):
            for k in range(4):
                rhs = sil[:, ts(k, NTOK)] if g == 8 else feat[:, g, ts(k, NTOK)]
                for m in range(4):
                    nc.tensor.matmul(
                        pms[m][:], w[:, g * 4 + k, m, :], rhs,
                        start=(gi == 0 and k == 0), stop=(gi == 8 and k == 3),
                    )
        for m in range(4):
            out_cb(m, pms[m])

    def kan_matmul_512_to_64(self, feat, sil, w):
        """-> psum (64, 256) after 36 matmuls. w (128, 36, 64)."""
        nc = self.nc
        pm = self.psum1.tile([64, NTOK], F32, tag="pk64")
        n = 0
        for g in [8] + list(range(8)):
            for k in range(4):
                rhs = sil[:, ts(k, NTOK)] if g == 8 else feat[:, g, ts(k, NTOK)]
                nc.tensor.matmul(
                    pm[:], w[:, g * 4 + k, :], rhs, start=(n == 0), stop=(n == 35)
                )
                n += 1
        return pm

    def kan_matmul_64_to_512(self, feat, sil, w, out_cb):
        """feat (64,8,256), sil (64,256), w (64,9,4,128)."""
        nc = self.nc
        pms = [
            self.psum4.tile([128, NTOK], F32, tag="pmm", name=f"pmm{m}")
            for m in range(4)
        ]
        gorder = [8] + list(range(8))
        for gi, g in enumerate(gorder):
            rhs = sil[:] if g == 8 else feat[:, g, :]
            for m in range(4):
                nc.tensor.matmul(
                    pms[m][:], w[:, g, m, :], rhs, start=(gi == 0), stop=(gi == 8)
                )
        for m in range(4):
            out_cb(m, pms[m])

    # ---- layernorm ------------------------------------------------------ #
    def stats_from(self, srcs):
        """srcs: list of (b, ap) free-dim slabs covering each batch; emits
        Identity+Square accum passes and returns stats tile (128, n) with
        layout [sum, sumsq] per accum slot plus the slot->batch map."""
        nc = self.nc
        n = len(srcs)
        stats = self.tiny.tile([128, 2 * n], F32, name="stats")
        for j, (b, sl) in enumerate(srcs):
            scr1 = self.scr.tile(list(sl.shape), F32, tag="sqscr", name=f"scr1_{j}")
            nc.scalar.activation(
                scr1[:], sl, AF.Identity, accum_out=stats[:, 2 * j : 2 * j + 1]
            )
            sqr = self.scr.tile(list(sl.shape), F32, tag="sqscr", name=f"sqr_{j}")
            nc.scalar.activation(
                sqr[:], sl, AF.Square, accum_out=stats[:, 2 * j + 1 : 2 * j + 2]
            )
        return stats

    def layernorm(self, xT, zname, lnw=None, lnb=None, stats=None, smap=None,
                  neng=None):
        """xT (128, 4, NTOK) fp32 -> z normalized per batch.  stats: tile
        (128, 2n) of [sum, sumsq] accum slots; smap[j] = batch of slot j
        (slots of one batch are summed)."""
        nc = self.nc
        if stats is None:
            stats = self.stats_from(
                [(b, xT[:, :, ts(b, C4)]) for b in range(BPC)]
            )
            smap = list(range(BPC))
        neng = neng or self.nc.gpsimd
        n2 = stats.shape[1]
        pstat = self.psum.tile([128, 128], F32, tag="ptr", name="pstat")[:, :n2]
        nc.tensor.matmul(pstat[:], self.ones[:], stats[:], start=True, stop=True)
        statsG = self.tiny.tile([128, n2], F32, name="statsG")
        nc.vector.tensor_scalar(
            out=statsG[:], in0=pstat[:], scalar1=INV_CNT, scalar2=None,
            op0=mybir.AluOpType.mult,
        )
        if len(smap) > BPC:
            # fold multiple slots per batch (pairwise into statsF)
            statsF = self.tiny.tile([128, 2 * BPC], F32, name="statsF")
            for b in range(BPC):
                idx = [j for j, bb in enumerate(smap) if bb == b]
                dst = statsF[:, 2 * b : 2 * b + 2]
                neng.tensor_add(
                    dst, statsG[:, 2 * idx[0] : 2 * idx[0] + 2],
                    statsG[:, 2 * idx[1] : 2 * idx[1] + 2],
                )
                for j in idx[2:]:
                    neng.tensor_add(dst, dst, statsG[:, 2 * j : 2 * j + 2])
        else:
            statsF = statsG
        mu = statsF[:, 0 : 2 * BPC : 2]
        e2 = statsF[:, 1 : 2 * BPC : 2]
        var = self.tiny.tile([128, BPC], F32)
        neng.tensor_mul(var[:], mu, mu)
        neng.tensor_sub(var[:], e2, var[:])
        a = self.tiny.tile([128, BPC], F32)
        neng.tensor_scalar_add(a[:], var[:], EPS)
        # y = rsqrt(a) by Newton from y0 = min(1/a, 1) (monotone from below)
        y = self.tiny.tile([128, BPC], F32)
        nc.vector.reciprocal(y[:], a[:])
        neng.tensor_scalar_min(y[:], y[:], 1.0)
        t = self.tiny.tile([128, BPC], F32)
        for _ in range(7):
            neng.tensor_mul(t[:], y[:], y[:])
            neng.tensor_mul(t[:], t[:], a[:])
            neng.tensor_scalar(
                out=t[:], in0=t[:], scalar1=-0.5, scalar2=1.5,
                op0=mybir.AluOpType.mult, op1=mybir.AluOpType.add,
            )
            neng.tensor_mul(y[:], y[:], t[:])
        musc = self.tiny.tile([128, BPC], F32)
        neng.tensor_mul(musc[:], mu, y[:])
        z = self.act.tile([128, 4, NTOK], F32, tag=zname)
        for b in range(BPC):
            nc.vector.tensor_scalar(
                out=z[:, :, ts(b, C4)], in0=xT[:, :, ts(b, C4)],
                scalar1=y[:, b : b + 1], scalar2=musc[:, b : b + 1],
                op0=mybir.AluOpType.mult, op1=mybir.AluOpType.subtract,
            )
        if lnw is not None:
            nc.vector.tensor_mul(z[:], z[:], lnw[:])
        if lnb is not None:
            nc.vector.tensor_add(z[:], z[:], lnb[:])
        return z

    # ---- gcn (folded) ---------------------------------------------------- #
    def gcn(self, tm_bf, wg, bias, bias_sc, yname, fin_gp=False):
        """tm_bf (128,4,NTOK) bf16; wg (128,4,4,128) bf16; bias (128,4) f32.
        Returns y (128,4,NTOK) f32 = gelu(tm @ Wg + b)."""
        nc = self.nc
        y = self.act.tile([128, 4, NTOK], F32, tag=yname)
        for m in range(4):
            pm = self.psum4.tile([128, NTOK], F32, tag="pmm")
            for k in range(4):
                nc.tensor.matmul(
                    pm[:], wg[:, k, m, :], tm_bf[:, k, :],
                    start=(k == 0), stop=(k == 3),
                )
            e = self.scr.tile([128, NTOK], F32, tag="erf")
            nc.scalar.activation(
                e[:], pm[:], AF.Erf, bias=bias_sc[:, m : m + 1], scale=ISQ2
            )
            if fin_gp:
                hb = self.scr.tile([128, NTOK], F32, tag="hb", name=f"hb{m}")
                nc.scalar.activation(
                    hb[:], pm[:], AF.Identity, bias=bias[:, m : m + 1]
                )
                t1 = self.scr.tile([128, NTOK], F32, tag="hb", name=f"gf{m}")
                nc.gpsimd.tensor_scalar_add(t1[:], e[:], 1.0)
                nc.gpsimd.tensor_mul(t1[:], t1[:], hb[:])
                nc.gpsimd.tensor_scalar(
                    out=y[:, m, :], in0=t1[:], scalar1=0.5, scalar2=None,
                    op0=mybir.AluOpType.mult,
                )
            else:
                nc.vector._custom_dve(
                    GELU_FIN2, out=y[:, m, :], in0=pm[:], in1=e[:], s0=0.5,
                    s1=bias[:, m : m + 1],
                )
        return y


def _emit(nc, ln_flags):
    """Emit the full per-core kernel.  ln_flags = (use_lnw1, use_lnb1,
    use_lnw2, use_lnb2) -- whether the TM layernorm affine params are
    non-trivial and must be applied."""
    use_lnw1, use_lnb1, use_lnw2, use_lnb2 = ln_flags
    dram = {}

    def din(name, shape, dt=BF16):
        dram[name] = nc.dram_tensor(name, shape, dt, kind="ExternalInput").ap()
        return dram[name]

    x_d = din("x_sh", (BPC, C4, T), F32)
    w_tm1k1 = din("w_tm1k1", (128, 36, 64))
    w_tm1k2 = din("w_tm1k2", (64, 9, 4, 128))
    w_k1 = din("w_k1", (128, 36, 4, 128))
    w_g1 = din("w_g1", (128, 4, 4, 128))
    b_g1 = din("b_g1", (128, 4, 2), F32)        # [:, :, 0]=b, [:, :, 1]=b/sqrt2
    w_tm2k1 = din("w_tm2k1", (128, 36, 64))
    w_tm2k2 = din("w_tm2k2", (64, 9, 4, 128))
    w_g2 = din("w_g2", (128, 4, 4, 128))
    b_g2 = din("b_g2", (128, 4, 2), F32)
    w_k2 = din("w_k2", (128, 36, 4, 128))
    ln1w_d = din("ln1w", (128, 4, NTOK)) if use_lnw1 else None
    ln1b_d = din("ln1b", (128, 4, NTOK)) if use_lnb1 else None
    ln2w_d = din("ln2w", (128, 4, NTOK)) if use_lnw2 else None
    ln2b_d = din("ln2b", (128, 4, NTOK)) if use_lnb2 else None
    out_d = nc.dram_tensor("out_sh", (BPC, C4, T), F32, kind="ExternalOutput").ap()

    with tile.TileContext(nc) as tc, ExitStack() as ctx:
        kb = _KB(nc, tc, ctx)
        wpool = ctx.enter_context(tc.tile_pool(name="weights", bufs=1))
        sync = nc.sync

        # ---- weight/input DMA (issue in consumption order) ----
        def wload(ap, shape, tag, dt=BF16):
            t = wpool.tile(list(shape), dt, tag=tag)
            sync.dma_start(t[:], ap)
            return t

        xN = kb.act.tile([C4, BPC, T], F32, tag="nat")
        x_r = x_d.rearrange("b p t -> p b t")
        for k in range(4):
            for b in range(BPC):
                sync.dma_start(
                    xN[:, b, ts(k, 128)], x_r[:, b, ts(k, 128)]
                )
        W_k1 = wload(w_k1, (128, 36, 4, 128), "wk_big")
        W_tm2k1 = wload(w_tm2k1, (128, 36, 64), "wtm2k1")
        W_tm2k2 = wload(w_tm2k2, (64, 9, 4, 128), "wtm2k2")
        W_g2 = wload(w_g2, (128, 4, 4, 128), "wg2")
        B_g2 = wload(b_g2, (128, 4, 2), "bg2", F32)
        W_tm1k1 = wload(w_tm1k1, (128, 36, 64), "wtm1k1")
        W_tm1k2 = wload(w_tm1k2, (64, 9, 4, 128), "wtm1k2")
        W_g1 = wload(w_g1, (128, 4, 4, 128), "wg1")
        B_g1 = wload(b_g1, (128, 4, 2), "bg1", F32)
        W_k2 = wload(w_k2, (128, 36, 4, 128), "wk_big")
        LN1W = wload(ln1w_d, (128, 4, NTOK), "ln1w") if use_lnw1 else None
        LN1B = wload(ln1b_d, (128, 4, NTOK), "ln1b") if use_lnb1 else None
        LN2W = wload(ln2w_d, (128, 4, NTOK), "ln2w") if use_lnw2 else None
        LN2B = wload(ln2b_d, (128, 4, NTOK), "ln2b") if use_lnb2 else None

        # ---- transpose x into T-layout ----
        xT = kb.act.tile([128, 4, NTOK], F32, tag="xT")
        for k in range(4):
            for b in range(BPC):
                pt = kb.psum.tile([128, 128], F32, tag="ptr")
                nc.tensor.transpose(pt[:], xN[:, b, ts(k, 128)], kb.ident[:])
                nc.scalar.copy(xT[:, k, ts(b, 128)], pt[:])

        # ---- TM1 stats from xN (ready before transposes finish) ----
        stats1 = kb.stats_from([(b, xN[:, b, :]) for b in range(BPC)])
        z1 = kb.layernorm(xT, "z", LN1W, LN1B, stats=stats1,
                          smap=list(range(BPC)))
        featX, silX = kb.kan_features(xT[:, :, :], 128, 4 * NTOK, "x", split=4)

        # ---- k1 matmuls -> cm (critical chain head) ----
        cm = kb.act.tile([128, 4, NTOK], F32, tag="cm")
        kb.kan_matmul_512(
            featX, silX, W_k1,
            lambda m, pm: nc.scalar.copy(cm[:, m, :], pm[:]),
        )

        # ---- TM1 kan chain + gcn1 (PE work emitted before the blocked
        #      tm2 matmuls so the in-order PE stream isn't inverted) ----
        featA, silA = kb.kan_features(z1[:, :, :], 128, 4 * NTOK, "a")
        p1 = kb.kan_matmul_512_to_64(featA, silA, W_tm1k1)
        z2 = kb.act.tile([64, NTOK], F32, tag="z64")
        nc.scalar.copy(z2[:], p1[:])
        featB, silB = kb.kan_features(z2[:, :], 64, NTOK, "b")
        tm1 = kb.bfa.tile([128, 4, NTOK], BF16, tag="tm")
        kb.kan_matmul_64_to_512(
            featB, silB, W_tm1k2,
            lambda m, pm: nc.vector.tensor_add(tm1[:, m, :], xT[:, m, :], pm[:]),
        )
        y1 = kb.gcn(tm1, W_g1, B_g1[:, :, 0], B_g1[:, :, 1], "y", fin_gp=True)

        # ---- TM2 on cm ----
        z3 = kb.layernorm(cm, "z", LN2W, LN2B)
        featC, silC = kb.kan_features(z3[:, :, :], 128, 4 * NTOK, "c")
        p2 = kb.kan_matmul_512_to_64(featC, silC, W_tm2k1)
        z4 = kb.act.tile([64, NTOK], F32, tag="z64")
        nc.scalar.copy(z4[:], p2[:])
        featD, silD = kb.kan_features(z4[:, :], 64, NTOK, "d")
        tm2 = kb.bfa.tile([128, 4, NTOK], BF16, tag="tm")
        kb.kan_matmul_64_to_512(
            featD, silD, W_tm2k2,
            lambda m, pm: nc.vector.tensor_add(tm2[:, m, :], cm[:, m, :], pm[:]),
        )
        y2 = kb.gcn(tm2, W_g2, B_g2[:, :, 0], B_g2[:, :, 1], "y")

        # ---- k2 on y2, final add, transpose out ----
        featY, silY = kb.kan_features(y2[:, :, :], 128, 4 * NTOK, "y")
        outT = kb.act.tile([128, 4, NTOK], F32, tag="outT")
        kb.kan_matmul_512(
            featY, silY, W_k2,
            lambda m, pm: nc.vector.tensor_add(outT[:, m, :], y1[:, m, :], pm[:]),
        )

        outN = kb.act.tile([C4, BPC, T], F32, tag="nat")
        out_r = out_d.rearrange("b p t -> p b t")
        for m in range(4):
            for b in range(BPC):
                pt = kb.psum.tile([128, 128], F32, tag="ptr")
                nc.tensor.transpose(pt[:], outT[:, m, ts(b, 128)], kb.ident[:])
                nc.scalar.copy(outN[:, b, ts(m, 128)], pt[:])
            sync.dma_start(
                out_r[:, :, ts(m, 128)], outN[:, :, ts(m, 128)]
            )

    return dram


def _build(ln_flags):
    key = ln_flags
    if key in _COMPILED:
        return _COMPILED[key]
    nc = bacc.Bacc("TRN2", target_bir_lowering=False, debug=False)
    _emit(nc, ln_flags)
    nc.compile()
    _COMPILED[key] = nc
    return nc


# --------------------------------------------------------------------------
# host-side weight preparation
# --------------------------------------------------------------------------
def _prep_kan_512(base_w, spline_w):
    """base_w (O,512), spline_w (O,512,8) -> (128, 36, O) or (128,36,4,128)."""
    O = base_w.shape[0]
    w = np.empty((128, 36, O), np.float32)
    for g in range(8):
        for k in range(4):
            # rows p -> channel 128k+p, feature basis g (x 1/6)
            w[:, g * 4 + k, :] = spline_w[:, k * 128 : (k + 1) * 128, g].T / 6.0
    for k in range(4):
        w[:, 32 + k, :] = base_w[:, k * 128 : (k + 1) * 128].T
    w = w.astype(BF)
    if O == 512:
        return np.ascontiguousarray(w.reshape(128, 36, 4, 128))
    return np.ascontiguousarray(w)


def _prep_kan_64(base_w, spline_w):
    """base_w (512,64), spline_w (512,64,8) -> (64, 9, 4, 128)."""
    w = np.empty((64, 9, 4, 128), np.float32)
    for g in range(8):
        for m in range(4):
            w[:, g, m, :] = spline_w[m * 128 : (m + 1) * 128, :, g].T / 6.0
    for m in range(4):
        w[:, 8, m, :] = base_w[m * 128 : (m + 1) * 128, :].T
    return np.ascontiguousarray(w.astype(BF))


def _prep_gcn(gw, gb):
    """gw (512, 1536) -> folded (128,4,4,128) bf16 lhsT; gb -> (128,4,2) f32."""
    Wf = gw[:, :512] + gw[:, 512:1024] + gw[:, 1024:]
    w = np.empty((128, 4, 4, 128), np.float32)
    for k in range(4):
        for m in range(4):
            w[:, k, m, :] = Wf[m * 128 : (m + 1) * 128, k * 128 : (k + 1) * 128].T
    b = np.empty((128, 4, 2), np.float32)
    b[:, :, 0] = gb.reshape(4, 128).T
    b[:, :, 1] = b[:, :, 0] * ISQ2
    return np.ascontiguousarray(w.astype(BF)), np.ascontiguousarray(b)


def _ln_plane(a):
    """ln param (512, 128) -> (128, 4, NTOK) bf16 duplicated over batches."""
    p = np.empty((128, 4, NTOK), np.float32)
    for k in range(4):
        for b in range(BPC):
            p[:, k, b * C4 : (b + 1) * C4] = a[k * 128 : (k + 1) * 128, :]
    return np.ascontiguousarray(p.astype(BF))


def kernel(**inputs):
    i = {k: np.asarray(v) for k, v in inputs.items()}
    use_lnw1 = not np.all(i["tm1_ln_w"] == 1.0)
    use_lnb1 = not np.all(i["tm1_ln_b"] == 0.0)
    use_lnw2 = not np.all(i["tm_ln_w"] == 1.0)
    use_lnb2 = not np.all(i["tm_ln_b"] == 0.0)
    ln_flags = (use_lnw1, use_lnb1, use_lnw2, use_lnb2)
    nc = _build(ln_flags)

    w_tm1k1 = _prep_kan_512(i["tm1_k1_base"], i["tm1_k1_spline"])
    w_tm1k2 = _prep_kan_64(i["tm1_k2_base"], i["tm1_k2_spline"])
    w_k1 = _prep_kan_512(i["k1_base"], i["k1_spline"])
    w_g1, b_g1 = _prep_gcn(i["g1_w"], i["g1_b"])
    w_tm2k1 = _prep_kan_512(i["tm_k1_base"], i["tm_k1_spline"])
    w_tm2k2 = _prep_kan_64(i["tm_k2_base"], i["tm_k2_spline"])
    w_g2, b_g2 = _prep_gcn(i["g2_w"], i["g2_b"])
    w_k2 = _prep_kan_512(i["k2_base"], i["k2_spline"])

    shared = dict(
        w_tm1k1=w_tm1k1, w_tm1k2=w_tm1k2, w_k1=w_k1, w_g1=w_g1, b_g1=b_g1,
        w_tm2k1=w_tm2k1, w_tm2k2=w_tm2k2, w_g2=w_g2, b_g2=b_g2, w_k2=w_k2,
    )
    if use_lnw1:
        shared["ln1w"] = _ln_plane(i["tm1_ln_w"])
    if use_lnb1:
        shared["ln1b"] = _ln_plane(i["tm1_ln_b"])
    if use_lnw2:
        shared["ln2w"] = _ln_plane(i["tm_ln_w"])
    if use_lnb2:
        shared["ln2b"] = _ln_plane(i["tm_ln_b"])
    x = np.ascontiguousarray(i["x"], np.float32)
    in_maps = [
        {"x_sh": x[c * BPC : (c + 1) * BPC], **shared} for c in range(NCORES)
    ]
    res = run_bass_kernel_spmd(nc, in_maps, core_ids=list(range(NCORES)))
    out = np.empty((B, C4, T), np.float32)
    for c in range(NCORES):
        out[c * BPC : (c + 1) * BPC] = res.results[c]["out_sh"]
    return out
